# revision 1
# baseline (speedup 1.0000x reference)
"""Trainium2 Bass kernel for nn_AttGraphConvLayer.

Reference computation (per batch b):
    z   = nodes @ w                          [N, D]
    z1  = Cmat @ z ; z2 = Nmat @ z           [E, D] (one-hot gathers)
    att = leaky_relu(concat(z1, z2) @ attention)      [E, 1]
    scores = (Cmat^T * att^T) @ Nmat         [N, N]
    adj    = Cmat^T @ Nmat                   [N, N]
    logits = scores + (1 - adj) * (-1e9)
    out = leaky_relu(softmax(logits, -1) * adj @ z)   [N, D]

Key identities used (Cmat/Nmat are one-hot incidence matrices):
  * att_e = leaky(u[src_e] + v[dst_e]) with u = z @ a_top, v = z @ a_bot.
    Hence scores[n, m] = adj[n, m] * leaky(u[n] + v[m]) -- no [E,D]
    gathers and no scores matmul are needed at all; only the adjacency
    matmul (contraction over E) remains.
  * adj has 0/1 inputs, so the big [N,E]@[E,N] matmul is EXACT in bf16
    (products are 0/1, fp32 PSUM accumulation) and runs 4x faster than
    fp32 on the PE.
  * v = z @ a_bot = nodes @ (w @ a_bot): computed via a tiny on-device
    reduction (wb = sum_d w*a_bot) plus a PE matvec against nodes^T.

Sharding: 8 cores = 4 batches x 2 row-halves. All cores run the same
program; the host permutes each core's inputs so that the core's 512
output rows are always rows 0..511 (node axis permutation applied
consistently to nodes rows, Cmat columns and Nmat columns; the final
softmax/matmul over the m axis is permutation invariant).
"""

import sys

for _p in ("/opt/trn_rl_repo", "/root/.axon_site/_ro/trn_rl_repo"):
    if _p not in sys.path:
        sys.path.insert(0, _p)

import numpy as np

B, E, N, F, D = 4, 8192, 1024, 512, 512
H = N // 2          # rows per core
P = 128
KC = E // P         # 64 contraction chunks
ALPHA = 0.2
NEG = -1.0e9
N_CORES = 8

_compiled = None


def _build():
    import concourse.bacc as bacc
    import concourse.tile as tile
    import concourse.mybir as mybir
    from concourse.masks import make_identity

    dt = mybir.dt
    f32 = dt.float32
    bf16 = dt.bfloat16
    Alu = mybir.AluOpType
    Act = mybir.ActivationFunctionType

    nc = bacc.Bacc("TRN2", target_bir_lowering=False, debug=False,
                   num_devices=N_CORES)

    ch = nc.dram_tensor("ch", [E, H], f32, kind="ExternalInput").ap()
    nf = nc.dram_tensor("nf", [E, N], f32, kind="ExternalInput").ap()
    nodes = nc.dram_tensor("nodes", [N, F], f32, kind="ExternalInput").ap()
    w = nc.dram_tensor("w", [F, D], f32, kind="ExternalInput").ap()
    atop = nc.dram_tensor("atop", [1, D], f32, kind="ExternalInput").ap()
    abot = nc.dram_tensor("abot", [1, D], f32, kind="ExternalInput").ap()
    out = nc.dram_tensor("out", [H, D], f32, kind="ExternalOutput").ap()

    NC_N = N // P   # 8 node chunks
    NC_F = F // P   # 4 feature chunks
    NC_H = H // P   # 4 row chunks per core

    with tile.TileContext(nc) as tc:
        with tc.tile_pool(name="singles", bufs=1) as singles:
            ident = singles.tile([P, P], f32, name="ident")
            make_identity(nc, ident)

            nodes_sb = singles.tile([P, NC_N, F], f32, name="nodes_sb")
            nc.sync.dma_start(out=nodes_sb,
                              in_=nodes.rearrange("(c p) f -> p c f", p=P))
            w_sb = singles.tile([P, NC_F, D], f32, name="w_sb")
            nc.sync.dma_start(out=w_sb,
                              in_=w.rearrange("(c p) d -> p c d", p=P))
            atop_b = singles.tile([P, D], f32, name="atop_b")
            nc.sync.dma_start(out=atop_b, in_=atop.to_broadcast([P, D]))
            abot_b = singles.tile([P, D], f32, name="abot_b")
            nc.sync.dma_start(out=abot_b, in_=abot.to_broadcast([P, D]))

            # ---- nodes^T (f on partitions) via PE transpose ----
            nodesT_sb = singles.tile([P, NC_F, N], f32, name="nodesT_sb")
            with tc.tile_pool(name="tp_ps", bufs=4, space="PSUM") as tp_ps:
                for cn in range(NC_N):
                    for cf in range(NC_F):
                        pt = tp_ps.tile([P, P], f32, name=f"tp_{cn}_{cf}",
                                        tag="tp")
                        nc.tensor.transpose(
                            pt, nodes_sb[:, cn, cf * P:(cf + 1) * P], ident)
                        eng = nc.vector if (cn + cf) % 2 == 0 else nc.scalar
                        if eng is nc.vector:
                            eng.tensor_copy(
                                nodesT_sb[:, cf, cn * P:(cn + 1) * P], pt)
                        else:
                            eng.copy(
                                nodesT_sb[:, cf, cn * P:(cn + 1) * P], pt)

                # ---- z = nodes @ w (fp32), z_sb[p, c, d], n = c*128+p ----
                z_sb = singles.tile([P, NC_N, D], f32, name="z_sb")
                with tc.tile_pool(name="z_ps", bufs=2, space="PSUM") as z_ps:
                    for cn in range(NC_N):
                        zp = z_ps.tile([P, D], f32, name=f"zp_{cn}", tag="zp")
                        for cf in range(NC_F):
                            nc.tensor.matmul(
                                zp,
                                lhsT=nodesT_sb[:, cf, cn * P:(cn + 1) * P],
                                rhs=w_sb[:, cf, :],
                                start=(cf == 0), stop=(cf == NC_F - 1))
                        if cn % 2 == 0:
                            nc.vector.tensor_copy(z_sb[:, cn, :], zp)
                        else:
                            nc.scalar.copy(z_sb[:, cn, :], zp)

                # ---- u[n] = sum_d z[n,d] * a_top[d]  (DVE) ----
                u_sb = singles.tile([P, NC_N], f32, name="u_sb")
                wb_sb = singles.tile([P, NC_F], f32, name="wb_sb")
                with tc.tile_pool(name="uscr", bufs=2) as uscr:
                    for cn in range(NC_N):
                        us = uscr.tile([P, D], f32, name=f"us_{cn}", tag="us")
                        nc.vector.tensor_mul(us, z_sb[:, cn, :], atop_b)
                        nc.vector.tensor_reduce(
                            u_sb[:, cn:cn + 1], us,
                            axis=mybir.AxisListType.X, op=Alu.add)
                    # wb[f] = sum_d w[f,d] * a_bot[d]
                    for cf in range(NC_F):
                        ws = uscr.tile([P, D], f32, name=f"ws_{cf}", tag="us")
                        nc.vector.tensor_mul(ws, w_sb[:, cf, :], abot_b)
                        nc.vector.tensor_reduce(
                            wb_sb[:, cf:cf + 1], ws,
                            axis=mybir.AxisListType.X, op=Alu.add)

                # ---- v[m] = sum_f nodes[m,f] * wb[f]  (PE matvec) ----
                v_row = singles.tile([1, N], f32, name="v_row")
                with tc.tile_pool(name="v_ps", bufs=2, space="PSUM") as v_ps:
                    for jm in range(2):
                        vp = v_ps.tile([1, 512], f32, name=f"vp_{jm}",
                                       tag="vp")
                        for cf in range(NC_F):
                            nc.tensor.matmul(
                                vp,
                                lhsT=wb_sb[:, cf:cf + 1],
                                rhs=nodesT_sb[:, cf, jm * 512:(jm + 1) * 512],
                                start=(cf == 0), stop=(cf == NC_F - 1))
                        nc.vector.tensor_copy(
                            v_row[:, jm * 512:(jm + 1) * 512], vp)
                V_bc = singles.tile([P, N], f32, name="V_bc")
                nc.gpsimd.partition_broadcast(V_bc, v_row)

            # ---- adjacency matmul: adj = ch^T @ nf over E, bf16 exact ----
            adj_sb = singles.tile([P, NC_H, N], f32, name="adj_sb")
            with tc.tile_pool(name="adj_ps", bufs=1, space="PSUM") as adj_ps, \
                 tc.tile_pool(name="cstr", bufs=4) as cstr, \
                 tc.tile_pool(name="nstr", bufs=4) as nstr, \
                 tc.tile_pool(name="cbstr", bufs=4) as cbstr, \
                 tc.tile_pool(name="nbstr", bufs=4) as nbstr:
                adj_p = {}
                for r in range(NC_H):
                    for j in range(2):
                        adj_p[(r, j)] = adj_ps.tile(
                            [P, 512], f32, name=f"adj_{r}_{j}",
                            tag=f"adj_{r}_{j}")
                for k in range(KC):
                    ct = cstr.tile([P, H], f32, name=f"ct_{k}", tag="ct")
                    nc.sync.dma_start(out=ct, in_=ch[k * P:(k + 1) * P, :])
                    nt = nstr.tile([P, N], f32, name=f"nt_{k}", tag="nt")
                    nc.sync.dma_start(out=nt, in_=nf[k * P:(k + 1) * P, :])
                    cb = cbstr.tile([P, H], bf16, name=f"cb_{k}", tag="cb")
                    nb = nbstr.tile([P, N], bf16, name=f"nb_{k}", tag="nb")
                    if k % 2 == 0:
                        nc.vector.tensor_copy(cb, ct)
                        nc.scalar.copy(nb, nt)
                    else:
                        nc.scalar.copy(cb, ct)
                        nc.vector.tensor_copy(nb, nt)
                    for r in range(NC_H):
                        for j in range(2):
                            nc.tensor.matmul(
                                adj_p[(r, j)],
                                lhsT=cb[:, r * P:(r + 1) * P],
                                rhs=nb[:, j * 512:(j + 1) * 512],
                                start=(k == 0), stop=(k == KC - 1))
                # evict adj to SBUF
                for r in range(NC_H):
                    for j in range(2):
                        if (r + j) % 2 == 0:
                            nc.vector.tensor_copy(
                                adj_sb[:, r, j * 512:(j + 1) * 512],
                                adj_p[(r, j)])
                        else:
                            nc.scalar.copy(
                                adj_sb[:, r, j * 512:(j + 1) * 512],
                                adj_p[(r, j)])

            # ---- logits/softmax per row chunk ----
            # P_mat = leaky(u[n] + v[m]); logits = adj*P_mat + (adj-1)*1e9
            # (exact: for adj==1 the +(adj-1)*1e9 term is exactly 0)
            expadj_sb = singles.tile([P, NC_H, N], f32, name="expadj_sb")
            rcp_sb = singles.tile([P, NC_H], f32, name="rcp_sb")
            with tc.tile_pool(name="pscr", bufs=3) as pscr, \
                 tc.tile_pool(name="sml", bufs=4) as sml:
                for r in range(NC_H):
                    adj_r = adj_sb[:, r, :]
                    t_uv = pscr.tile([P, N], f32, name=f"tuv_{r}", tag="tuv")
                    # t_uv = V + u (ACT identity with per-partition bias)
                    nc.scalar.activation(t_uv, V_bc, Act.Identity,
                                         bias=u_sb[:, r:r + 1], scale=1.0)
                    p_t = pscr.tile([P, N], f32, name=f"pt_{r}", tag="ptl")
                    # leaky = max(x, 0.2x)
                    nc.vector.scalar_tensor_tensor(
                        out=p_t, in0=t_uv, scalar=ALPHA, in1=t_uv,
                        op0=Alu.mult, op1=Alu.max)
                    m1 = pscr.tile([P, N], f32, name=f"m1_{r}", tag="m1")
                    nc.vector.tensor_mul(m1, adj_r, p_t)
                    s19 = pscr.tile([P, N], f32, name=f"s19_{r}", tag="s19")
                    nc.vector.tensor_scalar(
                        out=s19, in0=adj_r, scalar1=-1.0, scalar2=-NEG,
                        op0=Alu.add, op1=Alu.mult)
                    lg = pscr.tile([P, N], f32, name=f"lg_{r}", tag="lg")
                    nc.vector.tensor_add(lg, m1, s19)
                    nmx = sml.tile([P, 1], f32, name=f"nmx_{r}", tag="nmx")
                    nc.vector.tensor_reduce(
                        nmx, lg, axis=mybir.AxisListType.X, op=Alu.max,
                        negate=True)
                    e_t = pscr.tile([P, N], f32, name=f"et_{r}", tag="et")
                    zr = sml.tile([P, 1], f32, name=f"zr_{r}", tag="zr")
                    nc.scalar.activation(e_t, lg, Act.Exp,
                                         bias=nmx[:, 0:1], scale=1.0,
                                         accum_out=zr)
                    nc.vector.tensor_mul(expadj_sb[:, r, :], e_t, adj_r)
                    nc.vector.reciprocal(rcp_sb[:, r:r + 1], zr)

            # ---- transpose expadj, final matmul, scale+leaky, store ----
            eT_sb = singles.tile([P, NC_N, H], f32, name="eT_sb")
            with tc.tile_pool(name="tp2_ps", bufs=4, space="PSUM") as tp2_ps:
                for r in range(NC_H):
                    for cm in range(NC_N):
                        pt2 = tp2_ps.tile([P, P], f32, name=f"tp2_{r}_{cm}",
                                          tag="tp2")
                        nc.tensor.transpose(
                            pt2, expadj_sb[:, r, cm * P:(cm + 1) * P], ident)
                        if (r + cm) % 2 == 0:
                            nc.vector.tensor_copy(
                                eT_sb[:, cm, r * P:(r + 1) * P], pt2)
                        else:
                            nc.scalar.copy(
                                eT_sb[:, cm, r * P:(r + 1) * P], pt2)

            out_r = out.rearrange("(r p) d -> p r d", p=P)
            with tc.tile_pool(name="o_ps", bufs=2, space="PSUM") as o_ps, \
                 tc.tile_pool(name="oscr", bufs=3) as oscr:
                for r in range(NC_H):
                    op = o_ps.tile([P, D], f32, name=f"op_{r}", tag="op")
                    for cm in range(NC_N):
                        nc.tensor.matmul(
                            op,
                            lhsT=eT_sb[:, cm, r * P:(r + 1) * P],
                            rhs=z_sb[:, cm, :],
                            start=(cm == 0), stop=(cm == NC_N - 1))
                    o_t = oscr.tile([P, D], f32, name=f"ot_{r}", tag="ot")
                    nc.vector.tensor_scalar(
                        out=o_t, in0=op, scalar1=rcp_sb[:, r:r + 1],
                        scalar2=None, op0=Alu.mult)
                    o_l = oscr.tile([P, D], f32, name=f"ol_{r}", tag="ol")
                    nc.vector.scalar_tensor_tensor(
                        out=o_l, in0=o_t, scalar=ALPHA, in1=o_t,
                        op0=Alu.mult, op1=Alu.max)
                    nc.sync.dma_start(out=out_r[:, r, :], in_=o_l)

    nc.compile()
    return nc


def _get_compiled():
    global _compiled
    if _compiled is None:
        _compiled = _build()
    return _compiled


def _in_maps(nodes, Cmat, Nmat, w, attention):
    nodes = np.asarray(nodes, dtype=np.float32)
    Cmat = np.asarray(Cmat, dtype=np.float32)
    Nmat = np.asarray(Nmat, dtype=np.float32)
    w = np.ascontiguousarray(np.asarray(w, dtype=np.float32))
    attention = np.asarray(attention, dtype=np.float32)
    atop = np.ascontiguousarray(attention[:D, 0][None, :])
    abot = np.ascontiguousarray(attention[D:, 0][None, :])
    maps = []
    for core in range(N_CORES):
        b, h = divmod(core, 2)
        lo, hi = h * H, (h + 1) * H
        if h == 0:
            nodes_p = np.ascontiguousarray(nodes[b])
            nf_p = np.ascontiguousarray(Nmat[b])
        else:
            nodes_p = np.concatenate([nodes[b, lo:hi], nodes[b, :lo]], axis=0)
            nf_p = np.concatenate([Nmat[b][:, lo:hi], Nmat[b][:, :lo]],
                                  axis=1)
        maps.append({
            "ch": np.ascontiguousarray(Cmat[b][:, lo:hi]),
            "nf": np.ascontiguousarray(nf_p),
            "nodes": np.ascontiguousarray(nodes_p),
            "w": w,
            "atop": atop,
            "abot": abot,
        })
    return maps


def kernel(nodes, Cmat, Nmat, mask, w, attention, _trace=False, _tmpdir=None):
    from concourse.bass_utils import run_bass_kernel_spmd

    nc = _get_compiled()
    maps = _in_maps(nodes, Cmat, Nmat, w, attention)
    res = run_bass_kernel_spmd(nc, maps, list(range(N_CORES)),
                               trace=_trace, tmpdir=_tmpdir)
    full = np.empty((B, N, D), dtype=np.float32)
    for core in range(N_CORES):
        b, h = divmod(core, 2)
        full[b, h * H:(h + 1) * H, :] = res.results[core]["out"]
    if _trace:
        return full, res
    return full


if __name__ == "__main__":
    # quick self-check with random one-hot inputs
    rng = np.random.default_rng(0)
    src = rng.integers(0, N, (B, E))
    dst = rng.integers(0, N, (B, E))
    Cm = np.eye(N, dtype=np.float32)[src]
    Nm = np.eye(N, dtype=np.float32)[dst]
    nodes = rng.standard_normal((B, N, F)).astype(np.float32)
    w = (rng.standard_normal((F, D)) * 0.05).astype(np.float32)
    att = (rng.standard_normal((2 * D, 1)) * 0.05).astype(np.float32)
    mask = np.ones((B, N, N), dtype=bool)
    got = kernel(nodes, Cm, Nm, mask, w, att)
    print("kernel ran, output shape", got.shape)


# revision 2
# speedup vs baseline: 1.5519x; 1.5519x over previous
"""Trainium2 Bass kernel for nn_AttGraphConvLayer.

Reference computation (per batch b):
    z   = nodes @ w                          [N, D]
    z1  = Cmat @ z ; z2 = Nmat @ z           [E, D] (one-hot gathers)
    att = leaky_relu(concat(z1, z2) @ attention)      [E, 1]
    scores = (Cmat^T * att^T) @ Nmat         [N, N]
    adj    = Cmat^T @ Nmat                   [N, N]
    logits = scores + (1 - adj) * (-1e9)
    out = leaky_relu(softmax(logits, -1) * adj @ z)   [N, D]

Key identities used (Cmat/Nmat are one-hot incidence matrices):
  * att_e = leaky(u[src_e] + v[dst_e]) with u = z @ a_top, v = z @ a_bot.
    Hence scores[n, m] = adj[n, m] * leaky(u[n] + v[m]) -- no [E,D]
    gathers and no scores matmul are needed at all; only the adjacency
    matmul (contraction over E) remains.
  * adj has 0/1 inputs, so the big [N,E]@[E,N] matmul is EXACT in bf16
    (products are 0/1, fp32 PSUM accumulation) and runs 4x faster than
    fp32 on the PE. The 0/1 incidence matrices are shipped as bf16 from
    the host (exact, halves the DMA bytes, no on-device casts).
  * v = z @ a_bot = nodes @ (w @ a_bot): computed via a tiny on-device
    reduction (wb = sum_d w*a_bot) plus a PE matvec against nodes^T.

Sharding: 8 cores = 4 batches x 2 row-halves (graph partitioning by
source node). A core's output rows n in [h*512,(h+1)*512) only receive
contributions from edges with src in that range, so the host ships each
core only those ~4096 edges (padded with all-zero rows to a fixed 4608).
All cores run the same program; the host permutes the node axis per core
so the core's 512 output rows are always rows 0..511 (applied
consistently to nodes rows, Cmat columns and Nmat columns; softmax and
the final contraction over the m axis are permutation invariant).
"""

import sys

for _p in ("/opt/trn_rl_repo", "/root/.axon_site/_ro/trn_rl_repo"):
    if _p not in sys.path:
        sys.path.insert(0, _p)

import numpy as np

B, E, N, F, D = 4, 8192, 1024, 512, 512
H = N // 2          # rows per core
P = 128
ESEL = 4608         # padded per-core edge count (|Binom(8192,.5)-4096| > 512
                    # has probability ~1e-29; asserted at runtime)
KC = ESEL // P      # 36 contraction chunks
ALPHA = 0.2
NEG = -1.0e9
N_CORES = 8

_compiled = None


def _build():
    import concourse.bacc as bacc
    import concourse.tile as tile
    import concourse.mybir as mybir
    from concourse.masks import make_identity

    dt = mybir.dt
    f32 = dt.float32
    bf16 = dt.bfloat16
    Alu = mybir.AluOpType
    Act = mybir.ActivationFunctionType

    nc = bacc.Bacc("TRN2", target_bir_lowering=False, debug=False,
                   num_devices=N_CORES)

    ch = nc.dram_tensor("ch", [ESEL, H], bf16, kind="ExternalInput").ap()
    nf = nc.dram_tensor("nf", [ESEL, N], bf16, kind="ExternalInput").ap()
    nodes = nc.dram_tensor("nodes", [N, F], f32, kind="ExternalInput").ap()
    w = nc.dram_tensor("w", [F, D], f32, kind="ExternalInput").ap()
    atop = nc.dram_tensor("atop", [1, D], f32, kind="ExternalInput").ap()
    abot = nc.dram_tensor("abot", [1, D], f32, kind="ExternalInput").ap()
    out = nc.dram_tensor("out", [H, D], f32, kind="ExternalOutput").ap()

    NC_N = N // P   # 8 node chunks
    NC_F = F // P   # 4 feature chunks
    NC_H = H // P   # 4 row chunks per core

    nodes_r = nodes.rearrange("(c p) f -> p c f", p=P)

    with tile.TileContext(nc) as tc:
        with tc.tile_pool(name="singles", bufs=1) as singles:
            ident = singles.tile([P, P], f32, name="ident")
            make_identity(nc, ident)

            w_sb = singles.tile([P, NC_F, D], f32, name="w_sb")
            nc.sync.dma_start(out=w_sb,
                              in_=w.rearrange("(c p) d -> p c d", p=P))
            atop_b = singles.tile([P, D], f32, name="atop_b")
            nc.sync.dma_start(out=atop_b, in_=atop.to_broadcast([P, D]))
            abot_b = singles.tile([P, D], f32, name="abot_b")
            nc.sync.dma_start(out=abot_b, in_=abot.to_broadcast([P, D]))

            # ---- prologue, pipelined per node chunk:
            # load nodes chunk -> 4 PE transposes -> z matmul -> u reduce
            nodes_sb = singles.tile([P, NC_N, F], f32, name="nodes_sb")
            nodesT_sb = singles.tile([P, NC_F, N], f32, name="nodesT_sb")
            z_sb = singles.tile([P, NC_N, D], f32, name="z_sb")
            u_sb = singles.tile([P, NC_N], f32, name="u_sb")
            wb_sb = singles.tile([P, NC_F], f32, name="wb_sb")
            with tc.tile_pool(name="tp_ps", bufs=4, space="PSUM") as tp_ps, \
                 tc.tile_pool(name="z_ps", bufs=2, space="PSUM") as z_ps, \
                 tc.tile_pool(name="uscr", bufs=2) as uscr:
                for cn in range(NC_N):
                    nc.sync.dma_start(out=nodes_sb[:, cn, :],
                                      in_=nodes_r[:, cn, :])
                    for cf in range(NC_F):
                        pt = tp_ps.tile([P, P], f32, name=f"tp_{cn}_{cf}",
                                        tag="tp")
                        nc.tensor.transpose(
                            pt, nodes_sb[:, cn, cf * P:(cf + 1) * P], ident)
                        if (cn + cf) % 2 == 0:
                            nc.vector.tensor_copy(
                                nodesT_sb[:, cf, cn * P:(cn + 1) * P], pt)
                        else:
                            nc.scalar.copy(
                                nodesT_sb[:, cf, cn * P:(cn + 1) * P], pt)
                    zp = z_ps.tile([P, D], f32, name=f"zp_{cn}", tag="zp")
                    for cf in range(NC_F):
                        nc.tensor.matmul(
                            zp,
                            lhsT=nodesT_sb[:, cf, cn * P:(cn + 1) * P],
                            rhs=w_sb[:, cf, :],
                            start=(cf == 0), stop=(cf == NC_F - 1))
                    if cn % 2 == 0:
                        nc.vector.tensor_copy(z_sb[:, cn, :], zp)
                    else:
                        nc.scalar.copy(z_sb[:, cn, :], zp)
                    # u[n] = sum_d z[n,d] * a_top[d]
                    us = uscr.tile([P, D], f32, name=f"us_{cn}", tag="us")
                    nc.vector.tensor_mul(us, z_sb[:, cn, :], atop_b)
                    nc.vector.tensor_reduce(
                        u_sb[:, cn:cn + 1], us,
                        axis=mybir.AxisListType.X, op=Alu.add)
                # wb[f] = sum_d w[f,d] * a_bot[d]
                for cf in range(NC_F):
                    ws = uscr.tile([P, D], f32, name=f"ws_{cf}", tag="us")
                    nc.vector.tensor_mul(ws, w_sb[:, cf, :], abot_b)
                    nc.vector.tensor_reduce(
                        wb_sb[:, cf:cf + 1], ws,
                        axis=mybir.AxisListType.X, op=Alu.add)

                # v[m] = sum_f nodes[m,f] * wb[f]  (PE matvec)
                v_row = singles.tile([1, N], f32, name="v_row")
                for jm in range(2):
                    vp = z_ps.tile([1, 512], f32, name=f"vp_{jm}", tag="zp")
                    for cf in range(NC_F):
                        nc.tensor.matmul(
                            vp,
                            lhsT=wb_sb[:, cf:cf + 1],
                            rhs=nodesT_sb[:, cf, jm * 512:(jm + 1) * 512],
                            start=(cf == 0), stop=(cf == NC_F - 1))
                    nc.vector.tensor_copy(
                        v_row[:, jm * 512:(jm + 1) * 512], vp)
                V_bc = singles.tile([P, N], f32, name="V_bc")
                nc.gpsimd.partition_broadcast(V_bc, v_row)

            # ---- adjacency matmul: adj = ch^T @ nf over selected edges ----
            adj_sb = singles.tile([P, NC_H, N], f32, name="adj_sb")
            with tc.tile_pool(name="adj_ps", bufs=1, space="PSUM") as adj_ps, \
                 tc.tile_pool(name="cstr", bufs=4) as cstr, \
                 tc.tile_pool(name="nstr", bufs=4) as nstr:
                adj_p = {}
                for r in range(NC_H):
                    for j in range(2):
                        adj_p[(r, j)] = adj_ps.tile(
                            [P, 512], f32, name=f"adj_{r}_{j}",
                            tag=f"adj_{r}_{j}")
                for k in range(KC):
                    cb = cstr.tile([P, H], bf16, name=f"cb_{k}", tag="cb")
                    nc.sync.dma_start(out=cb, in_=ch[k * P:(k + 1) * P, :])
                    nb = nstr.tile([P, N], bf16, name=f"nb_{k}", tag="nb")
                    nc.sync.dma_start(out=nb, in_=nf[k * P:(k + 1) * P, :])
                    for r in range(NC_H):
                        for j in range(2):
                            nc.tensor.matmul(
                                adj_p[(r, j)],
                                lhsT=cb[:, r * P:(r + 1) * P],
                                rhs=nb[:, j * 512:(j + 1) * 512],
                                start=(k == 0), stop=(k == KC - 1))
                # evict adj to SBUF (frees PSUM for the output stage)
                for r in range(NC_H):
                    for j in range(2):
                        if (r + j) % 2 == 0:
                            nc.vector.tensor_copy(
                                adj_sb[:, r, j * 512:(j + 1) * 512],
                                adj_p[(r, j)])
                        else:
                            nc.scalar.copy(
                                adj_sb[:, r, j * 512:(j + 1) * 512],
                                adj_p[(r, j)])

            # ---- per row chunk: softmax -> transpose -> out matmul ----
            # P_mat = leaky(u[n] + v[m]); logits = adj*P_mat + (adj-1)*1e9
            # (exact: for adj==1 the +(adj-1)*1e9 term is exactly 0)
            eT_sb = singles.tile([P, NC_N, H], f32, name="eT_sb")
            out_r = out.rearrange("(r p) d -> p r d", p=P)
            with tc.tile_pool(name="pscr", bufs=2) as pscr, \
                 tc.tile_pool(name="sml", bufs=4) as sml, \
                 tc.tile_pool(name="tp2_ps", bufs=4, space="PSUM") as tp2_ps, \
                 tc.tile_pool(name="o_ps", bufs=2, space="PSUM") as o_ps, \
                 tc.tile_pool(name="oscr", bufs=2) as oscr:
                for r in range(NC_H):
                    adj_r = adj_sb[:, r, :]
                    t_uv = pscr.tile([P, N], f32, name=f"tuv_{r}", tag="tuv")
                    # t_uv = V + u (ACT identity with per-partition bias)
                    nc.scalar.activation(t_uv, V_bc, Act.Identity,
                                         bias=u_sb[:, r:r + 1], scale=1.0)
                    p_t = pscr.tile([P, N], f32, name=f"pt_{r}", tag="ptl")
                    # leaky = max(x, 0.2x)
                    nc.vector.scalar_tensor_tensor(
                        out=p_t, in0=t_uv, scalar=ALPHA, in1=t_uv,
                        op0=Alu.mult, op1=Alu.max)
                    m1 = pscr.tile([P, N], f32, name=f"m1_{r}", tag="m1")
                    nc.vector.tensor_mul(m1, adj_r, p_t)
                    s19 = pscr.tile([P, N], f32, name=f"s19_{r}", tag="s19")
                    nc.vector.tensor_scalar(
                        out=s19, in0=adj_r, scalar1=-1.0, scalar2=-NEG,
                        op0=Alu.add, op1=Alu.mult)
                    lg = pscr.tile([P, N], f32, name=f"lg_{r}", tag="lg")
                    nc.vector.tensor_add(lg, m1, s19)
                    nmx = sml.tile([P, 1], f32, name=f"nmx_{r}", tag="nmx")
                    nc.vector.tensor_reduce(
                        nmx, lg, axis=mybir.AxisListType.X, op=Alu.max,
                        negate=True)
                    e_t = pscr.tile([P, N], f32, name=f"et_{r}", tag="et")
                    zr = sml.tile([P, 1], f32, name=f"zr_{r}", tag="zr")
                    nc.scalar.activation(e_t, lg, Act.Exp,
                                         bias=nmx[:, 0:1], scale=1.0,
                                         accum_out=zr)
                    ea = pscr.tile([P, N], f32, name=f"ea_{r}", tag="ea")
                    nc.vector.tensor_mul(ea, e_t, adj_r)
                    rcp = sml.tile([P, 1], f32, name=f"rcp_{r}", tag="rcp")
                    nc.vector.reciprocal(rcp, zr)

                    # transpose expadj chunk into eT columns r*128..
                    for cm in range(NC_N):
                        pt2 = tp2_ps.tile([P, P], f32, name=f"tp2_{r}_{cm}",
                                          tag="tp2")
                        nc.tensor.transpose(
                            pt2, ea[:, cm * P:(cm + 1) * P], ident)
                        if (r + cm) % 2 == 0:
                            nc.vector.tensor_copy(
                                eT_sb[:, cm, r * P:(r + 1) * P], pt2)
                        else:
                            nc.scalar.copy(
                                eT_sb[:, cm, r * P:(r + 1) * P], pt2)

                    # out chunk = leaky(rcp * (expadj^T)^T @ z)
                    op = o_ps.tile([P, D], f32, name=f"op_{r}", tag="op")
                    for cm in range(NC_N):
                        nc.tensor.matmul(
                            op,
                            lhsT=eT_sb[:, cm, r * P:(r + 1) * P],
                            rhs=z_sb[:, cm, :],
                            start=(cm == 0), stop=(cm == NC_N - 1))
                    o_t = oscr.tile([P, D], f32, name=f"ot_{r}", tag="ot")
                    nc.vector.tensor_scalar(
                        out=o_t, in0=op, scalar1=rcp[:, 0:1],
                        scalar2=None, op0=Alu.mult)
                    o_l = oscr.tile([P, D], f32, name=f"ol_{r}", tag="ol")
                    nc.vector.scalar_tensor_tensor(
                        out=o_l, in0=o_t, scalar=ALPHA, in1=o_t,
                        op0=Alu.mult, op1=Alu.max)
                    nc.sync.dma_start(out=out_r[:, r, :], in_=o_l)

    nc.compile()
    return nc


def _get_compiled():
    global _compiled
    if _compiled is None:
        _compiled = _build()
    return _compiled


def _in_maps(nodes, Cmat, Nmat, w, attention):
    import ml_dtypes
    bf = ml_dtypes.bfloat16
    nodes = np.asarray(nodes, dtype=np.float32)
    Cmat = np.asarray(Cmat, dtype=np.float32)
    Nmat = np.asarray(Nmat, dtype=np.float32)
    w = np.ascontiguousarray(np.asarray(w, dtype=np.float32))
    attention = np.asarray(attention, dtype=np.float32)
    atop = np.ascontiguousarray(attention[:D, 0][None, :])
    abot = np.ascontiguousarray(attention[D:, 0][None, :])
    maps = []
    for core in range(N_CORES):
        b, h = divmod(core, 2)
        lo, hi = h * H, (h + 1) * H
        # edges whose source lies in this core's row half
        src_in_half = Cmat[b][:, lo:hi].any(axis=1)
        sel = np.nonzero(src_in_half)[0]
        assert len(sel) <= ESEL, f"edge shard overflow: {len(sel)} > {ESEL}"
        ch_sel = np.zeros((ESEL, H), dtype=bf)
        ch_sel[:len(sel)] = Cmat[b][sel][:, lo:hi].astype(bf)
        nf_sel = np.zeros((ESEL, N), dtype=bf)
        nf_b = Nmat[b][sel]
        if h == 0:
            nf_sel[:len(sel)] = nf_b.astype(bf)
            nodes_p = np.ascontiguousarray(nodes[b])
        else:
            nf_sel[:len(sel), :H] = nf_b[:, lo:hi].astype(bf)
            nf_sel[:len(sel), H:] = nf_b[:, :lo].astype(bf)
            nodes_p = np.concatenate([nodes[b, lo:hi], nodes[b, :lo]], axis=0)
        maps.append({
            "ch": ch_sel,
            "nf": nf_sel,
            "nodes": np.ascontiguousarray(nodes_p),
            "w": w,
            "atop": atop,
            "abot": abot,
        })
    return maps


def kernel(nodes, Cmat, Nmat, mask, w, attention, _trace=False, _tmpdir=None):
    from concourse.bass_utils import run_bass_kernel_spmd

    nc = _get_compiled()
    maps = _in_maps(nodes, Cmat, Nmat, w, attention)
    res = run_bass_kernel_spmd(nc, maps, list(range(N_CORES)),
                               trace=_trace, tmpdir=_tmpdir)
    full = np.empty((B, N, D), dtype=np.float32)
    for core in range(N_CORES):
        b, h = divmod(core, 2)
        full[b, h * H:(h + 1) * H, :] = res.results[core]["out"]
    if _trace:
        return full, res
    return full


if __name__ == "__main__":
    rng = np.random.default_rng(0)
    src = rng.integers(0, N, (B, E))
    dst = rng.integers(0, N, (B, E))
    Cm = np.eye(N, dtype=np.float32)[src]
    Nm = np.eye(N, dtype=np.float32)[dst]
    nodes = rng.standard_normal((B, N, F)).astype(np.float32)
    w = (rng.standard_normal((F, D)) * 0.05).astype(np.float32)
    att = (rng.standard_normal((2 * D, 1)) * 0.05).astype(np.float32)
    mask = np.ones((B, N, N), dtype=bool)
    got = kernel(nodes, Cm, Nm, mask, w, att)
    print("kernel ran, output shape", got.shape)


# revision 3
# speedup vs baseline: 1.5934x; 1.0267x over previous
"""Trainium2 Bass kernel for nn_AttGraphConvLayer.

Reference computation (per batch b):
    z   = nodes @ w                          [N, D]
    z1  = Cmat @ z ; z2 = Nmat @ z           [E, D] (one-hot gathers)
    att = leaky_relu(concat(z1, z2) @ attention)      [E, 1]
    scores = (Cmat^T * att^T) @ Nmat         [N, N]
    adj    = Cmat^T @ Nmat                   [N, N]
    logits = scores + (1 - adj) * (-1e9)
    out = leaky_relu(softmax(logits, -1) * adj @ z)   [N, D]

Key identities used (Cmat/Nmat are one-hot incidence matrices):
  * att_e = leaky(u[src_e] + v[dst_e]) with u = z @ a_top, v = z @ a_bot.
    Hence scores[n, m] = adj[n, m] * leaky(u[n] + v[m]) -- no [E,D]
    gathers and no scores matmul are needed at all; only the adjacency
    matmul (contraction over E) remains.
  * adj has 0/1 inputs, so the big [N,E]@[E,N] matmul is EXACT in fp8
    (e4m3; products are 0/1, fp32 PSUM accumulation) and runs ~8x
    faster than fp32 on the PE with perf_mode=DoubleRow (2 edges
    contracted per cell per cycle). The incidence matrices are shipped
    as fp8 from the host (exact, 4x less DMA, no on-device casts).
  * v = z @ a_bot = nodes @ (w @ a_bot): computed via a tiny on-device
    reduction (wb = sum_d w*a_bot) plus a PE matvec against nodes^T.
    nodes^T itself is shipped from the host (layout choice), removing
    all PE transposes from the prologue.

Sharding: 8 cores = 4 batches x 2 row-halves (graph partitioning by
source node). A core's output rows n in [h*512,(h+1)*512) only receive
contributions from edges with src in that range, so the host ships each
core only those ~4096 edges (padded with all-zero rows to a fixed 4608).
All cores run the same program; the host permutes the node axis per core
so the core's 512 output rows are always rows 0..511 (applied
consistently to nodes rows, Cmat columns and Nmat columns; softmax and
the final contraction over the m axis are permutation invariant).
"""

import sys

for _p in ("/opt/trn_rl_repo", "/root/.axon_site/_ro/trn_rl_repo"):
    if _p not in sys.path:
        sys.path.insert(0, _p)

import numpy as np

B, E, N, F, D = 4, 8192, 1024, 512, 512
H = N // 2          # rows per core
P = 128
ESEL = 4608         # padded per-core edge count (|Binom(8192,.5)-4096| > 512
                    # has probability ~1e-29; asserted at runtime)
GB = 6              # stream DMA batches
SG = ESEL // GB // P  # 6 sub-chunks of 128 edges per batch
ALPHA = 0.2
NEG = -1.0e9
N_CORES = 8

_compiled = None


def _build():
    import concourse.bacc as bacc
    import concourse.tile as tile
    import concourse.mybir as mybir
    from concourse.masks import make_identity

    dt = mybir.dt
    f32 = dt.float32
    fp8 = dt.float8e4
    Alu = mybir.AluOpType
    Act = mybir.ActivationFunctionType
    DR = mybir.MatmulPerfMode.DoubleRow

    nc = bacc.Bacc("TRN2", target_bir_lowering=False, debug=False,
                   num_devices=N_CORES)

    ch = nc.dram_tensor("ch", [ESEL, H], fp8, kind="ExternalInput").ap()
    nf = nc.dram_tensor("nf", [ESEL, N], fp8, kind="ExternalInput").ap()
    nodesT = nc.dram_tensor("nodesT", [F, N], f32, kind="ExternalInput").ap()
    w = nc.dram_tensor("w", [F, D], f32, kind="ExternalInput").ap()
    atop = nc.dram_tensor("atop", [1, D], f32, kind="ExternalInput").ap()
    abot = nc.dram_tensor("abot", [1, D], f32, kind="ExternalInput").ap()
    out = nc.dram_tensor("out", [H, D], f32, kind="ExternalOutput").ap()

    NC_N = N // P   # 8 node chunks
    NC_F = F // P   # 4 feature chunks
    NC_H = H // P   # 4 row chunks per core

    with tile.TileContext(nc) as tc:
        with tc.tile_pool(name="singles", bufs=1) as singles:
            # ---- prologue ----
            nT_sb = singles.tile([P, NC_F, N], f32, name="nT_sb")
            nc.sync.dma_start(out=nT_sb,
                              in_=nodesT.rearrange("(c p) n -> p c n", p=P))
            w_sb = singles.tile([P, NC_F, D], f32, name="w_sb")
            nc.sync.dma_start(out=w_sb,
                              in_=w.rearrange("(c p) d -> p c d", p=P))
            atop_b = singles.tile([P, D], f32, name="atop_b")
            nc.sync.dma_start(out=atop_b, in_=atop.to_broadcast([P, D]))
            abot_b = singles.tile([P, D], f32, name="abot_b")
            nc.sync.dma_start(out=abot_b, in_=abot.to_broadcast([P, D]))
            ident = singles.tile([P, P], f32, name="ident")
            make_identity(nc, ident)

            # z = nodes @ w (fp32); z_sb[p, c, d] holds z[c*128+p, d]
            z_sb = singles.tile([P, NC_N, D], f32, name="z_sb")
            u_sb = singles.tile([P, NC_H], f32, name="u_sb")
            wb_sb = singles.tile([P, NC_F], f32, name="wb_sb")
            with tc.tile_pool(name="z_ps", bufs=2, space="PSUM") as z_ps, \
                 tc.tile_pool(name="uscr", bufs=2) as uscr:
                for cn in range(NC_N):
                    zp = z_ps.tile([P, D], f32, name=f"zp_{cn}", tag="zp")
                    for cf in range(NC_F):
                        nc.tensor.matmul(
                            zp,
                            lhsT=nT_sb[:, cf, cn * P:(cn + 1) * P],
                            rhs=w_sb[:, cf, :],
                            start=(cf == 0), stop=(cf == NC_F - 1))
                    if cn % 2 == 0:
                        nc.vector.tensor_copy(z_sb[:, cn, :], zp)
                    else:
                        nc.scalar.copy(z_sb[:, cn, :], zp)
                    if cn < NC_H:
                        # u[n] = sum_d z[n,d] * a_top[d] (rows 0..511 only)
                        us = uscr.tile([P, D], f32, name=f"us_{cn}", tag="us")
                        nc.vector.tensor_mul(us, z_sb[:, cn, :], atop_b)
                        nc.vector.tensor_reduce(
                            u_sb[:, cn:cn + 1], us,
                            axis=mybir.AxisListType.X, op=Alu.add)
                # wb[f] = sum_d w[f,d] * a_bot[d]
                for cf in range(NC_F):
                    ws = uscr.tile([P, D], f32, name=f"ws_{cf}", tag="us")
                    nc.vector.tensor_mul(ws, w_sb[:, cf, :], abot_b)
                    nc.vector.tensor_reduce(
                        wb_sb[:, cf:cf + 1], ws,
                        axis=mybir.AxisListType.X, op=Alu.add)
                # v[m] = sum_f nodes[m,f] * wb[f] (PE matvec)
                v_row = singles.tile([1, N], f32, name="v_row")
                for jm in range(2):
                    vp = z_ps.tile([1, 512], f32, name=f"vp_{jm}", tag="zp")
                    for cf in range(NC_F):
                        nc.tensor.matmul(
                            vp,
                            lhsT=wb_sb[:, cf:cf + 1],
                            rhs=nT_sb[:, cf, jm * 512:(jm + 1) * 512],
                            start=(cf == 0), stop=(cf == NC_F - 1))
                    nc.vector.tensor_copy(
                        v_row[:, jm * 512:(jm + 1) * 512], vp)
                V_bc = singles.tile([P, N], f32, name="V_bc")
                nc.gpsimd.partition_broadcast(V_bc, v_row)

            # pT = leaky(u[n] + v[m]) for all row chunks -- independent of
            # the adjacency matmul, computed while the PE streams it
            pT_all = singles.tile([P, NC_H, N], f32, name="pT_all")
            with tc.tile_pool(name="puv", bufs=2) as puv:
                for r in range(NC_H):
                    t_uv = puv.tile([P, N], f32, name=f"tuv_{r}", tag="tuv")
                    nc.scalar.activation(t_uv, V_bc, Act.Identity,
                                         bias=u_sb[:, r:r + 1], scale=1.0)
                    nc.vector.scalar_tensor_tensor(
                        out=pT_all[:, r, :], in0=t_uv, scalar=ALPHA,
                        in1=t_uv, op0=Alu.mult, op1=Alu.max)

            # ---- adjacency matmul: adj = ch^T @ nf, fp8 DoubleRow ----
            adj_sb = singles.tile([P, NC_H, N], f32, name="adj_sb")
            with tc.tile_pool(name="adj_ps", bufs=1, space="PSUM") as adj_ps, \
                 tc.tile_pool(name="cstr", bufs=3) as cstr, \
                 tc.tile_pool(name="nstr", bufs=3) as nstr:
                adj_p = {}
                for r in range(NC_H):
                    for j in range(2):
                        adj_p[(r, j)] = adj_ps.tile(
                            [P, 512], f32, name=f"adj_{r}_{j}",
                            tag=f"adj_{r}_{j}")
                for g in range(GB):
                    rows = slice(g * SG * P, (g + 1) * SG * P)
                    cb = cstr.tile([P, SG, H], fp8, name=f"cb_{g}", tag="cb")
                    nc.sync.dma_start(
                        out=cb, in_=ch[rows, :].rearrange("(s p) h -> p s h",
                                                          p=P))
                    nb = nstr.tile([P, SG, N], fp8, name=f"nb_{g}", tag="nb")
                    nc.sync.dma_start(
                        out=nb, in_=nf[rows, :].rearrange("(s p) h -> p s h",
                                                          p=P))
                    for t in range(SG // 2):
                        ks = slice(2 * t, 2 * t + 2)
                        for r in range(NC_H):
                            for j in range(2):
                                nc.tensor.matmul(
                                    adj_p[(r, j)],
                                    lhsT=cb[:, ks, r * P:(r + 1) * P],
                                    rhs=nb[:, ks, j * 512:(j + 1) * 512],
                                    start=(g == 0 and t == 0),
                                    stop=(g == GB - 1 and t == SG // 2 - 1),
                                    perf_mode=DR)
                # evict adj to SBUF (frees PSUM for the output stage)
                for r in range(NC_H):
                    for j in range(2):
                        if (r + j) % 2 == 0:
                            nc.vector.tensor_copy(
                                adj_sb[:, r, j * 512:(j + 1) * 512],
                                adj_p[(r, j)])
                        else:
                            nc.scalar.copy(
                                adj_sb[:, r, j * 512:(j + 1) * 512],
                                adj_p[(r, j)])

            # ---- per row chunk: softmax -> transpose -> out matmul ----
            # logits = adj*pT + (adj-1)*1e9
            # (exact: for adj==1 the +(adj-1)*1e9 term is exactly 0)
            eT_sb = singles.tile([P, NC_N, H], f32, name="eT_sb")
            out_r = out.rearrange("(r p) d -> p r d", p=P)
            with tc.tile_pool(name="pscr", bufs=2) as pscr, \
                 tc.tile_pool(name="sml", bufs=4) as sml, \
                 tc.tile_pool(name="tp2_ps", bufs=4, space="PSUM") as tp2_ps, \
                 tc.tile_pool(name="o_ps", bufs=2, space="PSUM") as o_ps, \
                 tc.tile_pool(name="oscr", bufs=2) as oscr:
                for r in range(NC_H):
                    adj_r = adj_sb[:, r, :]
                    m1 = pscr.tile([P, N], f32, name=f"m1_{r}", tag="m1")
                    nc.vector.tensor_mul(m1, adj_r, pT_all[:, r, :])
                    s19 = pscr.tile([P, N], f32, name=f"s19_{r}", tag="s19")
                    nc.gpsimd.tensor_scalar(
                        out=s19, in0=adj_r, scalar1=-1.0, scalar2=-NEG,
                        op0=Alu.add, op1=Alu.mult)
                    lg = pscr.tile([P, N], f32, name=f"lg_{r}", tag="lg")
                    nc.vector.tensor_add(lg, m1, s19)
                    nmx = sml.tile([P, 1], f32, name=f"nmx_{r}", tag="nmx")
                    nc.vector.tensor_reduce(
                        nmx, lg, axis=mybir.AxisListType.X, op=Alu.max,
                        negate=True)
                    e_t = pscr.tile([P, N], f32, name=f"et_{r}", tag="et")
                    zr = sml.tile([P, 1], f32, name=f"zr_{r}", tag="zr")
                    nc.scalar.activation(e_t, lg, Act.Exp,
                                         bias=nmx[:, 0:1], scale=1.0,
                                         accum_out=zr)
                    ea = pscr.tile([P, N], f32, name=f"ea_{r}", tag="ea")
                    nc.vector.tensor_mul(ea, e_t, adj_r)
                    rcp = sml.tile([P, 1], f32, name=f"rcp_{r}", tag="rcp")
                    nc.vector.reciprocal(rcp, zr)

                    # transpose expadj chunk into eT columns r*128..
                    for cm in range(NC_N):
                        pt2 = tp2_ps.tile([P, P], f32, name=f"tp2_{r}_{cm}",
                                          tag="tp2")
                        nc.tensor.transpose(
                            pt2, ea[:, cm * P:(cm + 1) * P], ident)
                        if (r + cm) % 2 == 0:
                            nc.vector.tensor_copy(
                                eT_sb[:, cm, r * P:(r + 1) * P], pt2)
                        else:
                            nc.scalar.copy(
                                eT_sb[:, cm, r * P:(r + 1) * P], pt2)

                    # out chunk = leaky(rcp * (expadj^T)^T @ z)
                    op = o_ps.tile([P, D], f32, name=f"op_{r}", tag="op")
                    for cm in range(NC_N):
                        nc.tensor.matmul(
                            op,
                            lhsT=eT_sb[:, cm, r * P:(r + 1) * P],
                            rhs=z_sb[:, cm, :],
                            start=(cm == 0), stop=(cm == NC_N - 1))
                    o_t = oscr.tile([P, D], f32, name=f"ot_{r}", tag="ot")
                    nc.vector.tensor_scalar(
                        out=o_t, in0=op, scalar1=rcp[:, 0:1],
                        scalar2=None, op0=Alu.mult)
                    o_l = oscr.tile([P, D], f32, name=f"ol_{r}", tag="ol")
                    nc.vector.scalar_tensor_tensor(
                        out=o_l, in0=o_t, scalar=ALPHA, in1=o_t,
                        op0=Alu.mult, op1=Alu.max)
                    nc.sync.dma_start(out=out_r[:, r, :], in_=o_l)

    nc.compile()
    return nc


def _get_compiled():
    global _compiled
    if _compiled is None:
        _compiled = _build()
    return _compiled


def _in_maps(nodes, Cmat, Nmat, w, attention):
    import ml_dtypes
    f8 = ml_dtypes.float8_e4m3
    nodes = np.asarray(nodes, dtype=np.float32)
    Cmat = np.asarray(Cmat, dtype=np.float32)
    Nmat = np.asarray(Nmat, dtype=np.float32)
    w = np.ascontiguousarray(np.asarray(w, dtype=np.float32))
    attention = np.asarray(attention, dtype=np.float32)
    atop = np.ascontiguousarray(attention[:D, 0][None, :])
    abot = np.ascontiguousarray(attention[D:, 0][None, :])
    maps = []
    for core in range(N_CORES):
        b, h = divmod(core, 2)
        lo, hi = h * H, (h + 1) * H
        # edges whose source lies in this core's row half
        src_in_half = Cmat[b][:, lo:hi].any(axis=1)
        sel = np.nonzero(src_in_half)[0]
        assert len(sel) <= ESEL, f"edge shard overflow: {len(sel)} > {ESEL}"
        ch_sel = np.zeros((ESEL, H), dtype=f8)
        ch_sel[:len(sel)] = Cmat[b][sel][:, lo:hi].astype(f8)
        nf_sel = np.zeros((ESEL, N), dtype=f8)
        nf_b = Nmat[b][sel]
        if h == 0:
            nf_sel[:len(sel)] = nf_b.astype(f8)
            nodesT = nodes[b].T
        else:
            nf_sel[:len(sel), :H] = nf_b[:, lo:hi].astype(f8)
            nf_sel[:len(sel), H:] = nf_b[:, :lo].astype(f8)
            nodesT = np.concatenate([nodes[b, lo:hi], nodes[b, :lo]],
                                    axis=0).T
        maps.append({
            "ch": ch_sel,
            "nf": nf_sel,
            "nodesT": np.ascontiguousarray(nodesT),
            "w": w,
            "atop": atop,
            "abot": abot,
        })
    return maps


def kernel(nodes, Cmat, Nmat, mask, w, attention, _trace=False, _tmpdir=None):
    from concourse.bass_utils import run_bass_kernel_spmd

    nc = _get_compiled()
    maps = _in_maps(nodes, Cmat, Nmat, w, attention)
    res = run_bass_kernel_spmd(nc, maps, list(range(N_CORES)),
                               trace=_trace, tmpdir=_tmpdir)
    full = np.empty((B, N, D), dtype=np.float32)
    for core in range(N_CORES):
        b, h = divmod(core, 2)
        full[b, h * H:(h + 1) * H, :] = res.results[core]["out"]
    if _trace:
        return full, res
    return full


if __name__ == "__main__":
    rng = np.random.default_rng(0)
    src = rng.integers(0, N, (B, E))
    dst = rng.integers(0, N, (B, E))
    Cm = np.eye(N, dtype=np.float32)[src]
    Nm = np.eye(N, dtype=np.float32)[dst]
    nodes = rng.standard_normal((B, N, F)).astype(np.float32)
    w = (rng.standard_normal((F, D)) * 0.05).astype(np.float32)
    att = (rng.standard_normal((2 * D, 1)) * 0.05).astype(np.float32)
    mask = np.ones((B, N, N), dtype=bool)
    got = kernel(nodes, Cm, Nm, mask, w, att)
    print("kernel ran, output shape", got.shape)


# revision 9
# speedup vs baseline: 1.9554x; 1.2272x over previous
"""Trainium2 Bass kernel for nn_AttGraphConvLayer.

Reference computation (per batch b):
    z   = nodes @ w                          [N, D]
    z1  = Cmat @ z ; z2 = Nmat @ z           [E, D] (one-hot gathers)
    att = leaky_relu(concat(z1, z2) @ attention)      [E, 1]
    scores = (Cmat^T * att^T) @ Nmat         [N, N]
    adj    = Cmat^T @ Nmat                   [N, N]
    logits = scores + (1 - adj) * (-1e9)
    out = leaky_relu(softmax(logits, -1) * adj @ z)   [N, D]

Key identities used (Cmat/Nmat are one-hot incidence matrices):
  * att_e = leaky(u[src_e] + v[dst_e]) with u = z @ a_top, v = z @ a_bot.
    Hence scores[n, m] = adj[n, m] * leaky(u[n] + v[m]) -- no [E,D]
    gathers and no scores matmul are needed at all; only the adjacency
    matmul (contraction over E) remains.
  * adj has 0/1 inputs, so the adjacency matmul is EXACT in fp8 (e4m3;
    products are 0/1, fp32 PSUM accumulation) and runs ~8x faster than
    fp32 on the PE with perf_mode=DoubleRow (2 edges contracted per
    cell per cycle). The incidence matrices are shipped as fp8 from the
    host (exact, less DMA, no on-device casts).
  * v = z @ a_bot = nodes @ (w @ a_bot): computed via a tiny on-device
    reduction (wb = sum_d w*a_bot) plus a PE matvec against nodes^T.
    nodes^T itself is shipped from the host (layout choice), removing
    all PE transposes from the prologue.

Sharding: 8 cores = 4 batches x 2 row-halves (graph partitioning by
source node). A core's output rows n in [h*512,(h+1)*512) only receive
contributions from edges with src in that range, so the host ships each
core only those ~4096 edges, further grouped by 128-row source chunk
(each group padded with all-zero rows to a fixed 1280). Grouping makes
the one-hot source block only 128 columns wide, so each adjacency PSUM
tile needs just its own group's edges: 40 DoubleRow matmuls total.
All cores run the same program; the host permutes the node axis per core
so the core's 512 output rows are always rows 0..511 (applied
consistently to nodes rows, Cmat columns and Nmat columns; softmax and
the final contraction over the m axis are permutation invariant).
"""

import sys

for _p in ("/opt/trn_rl_repo", "/root/.axon_site/_ro/trn_rl_repo"):
    if _p not in sys.path:
        sys.path.insert(0, _p)

import numpy as np

B, E, N, F, D = 4, 8192, 1024, 512, 512
H = N // 2          # rows per core
P = 128
EPG = 1280          # padded edges per source-chunk group; group size is
                    # Binom(8192, 1/8): mean 1024, sd 30 -> 1280 is +8.5 sd
                    # (asserted at runtime)
NG = H // P         # 4 groups per core
ALPHA = 0.2
NEG = -1.0e9
N_CORES = 8

_compiled = None


def _build():
    import concourse.bacc as bacc
    import concourse.tile as tile
    import concourse.mybir as mybir
    from concourse.masks import make_identity

    dt = mybir.dt
    f32 = dt.float32
    fp8 = dt.float8e4
    Alu = mybir.AluOpType
    Act = mybir.ActivationFunctionType
    DR = mybir.MatmulPerfMode.DoubleRow

    nc = bacc.Bacc("TRN2", target_bir_lowering=False, debug=False,
                   num_devices=N_CORES)

    # edge groups: group r covers source rows r*128..(r+1)*128
    ch = nc.dram_tensor("ch", [NG, EPG, P], fp8, kind="ExternalInput").ap()
    nf = nc.dram_tensor("nf", [NG, EPG, N], fp8, kind="ExternalInput").ap()
    nodesT = nc.dram_tensor("nodesT", [F, N], f32, kind="ExternalInput").ap()
    w = nc.dram_tensor("w", [F, D], f32, kind="ExternalInput").ap()
    atop = nc.dram_tensor("atop", [1, D], f32, kind="ExternalInput").ap()
    abot = nc.dram_tensor("abot", [1, D], f32, kind="ExternalInput").ap()
    out = nc.dram_tensor("out", [H, D], f32, kind="ExternalOutput").ap()

    NC_N = N // P   # 8 node chunks
    NC_F = F // P   # 4 feature chunks
    NC_H = H // P   # 4 row chunks per core
    SG = EPG // P   # 10 sub-chunks of 128 edges per group

    with tile.TileContext(nc) as tc:
        with tc.tile_pool(name="singles", bufs=1) as singles:
            # ---- prologue ----
            nT_sb = singles.tile([P, NC_F, N], f32, name="nT_sb")
            nc.sync.dma_start(out=nT_sb,
                              in_=nodesT.rearrange("(c p) n -> p c n", p=P))
            w_sb = singles.tile([P, NC_F, D], f32, name="w_sb")
            nc.sync.dma_start(out=w_sb,
                              in_=w.rearrange("(c p) d -> p c d", p=P))
            atop_b = singles.tile([P, D], f32, name="atop_b")
            nc.sync.dma_start(out=atop_b, in_=atop.to_broadcast([P, D]))
            abot_b = singles.tile([P, D], f32, name="abot_b")
            nc.sync.dma_start(out=abot_b, in_=abot.to_broadcast([P, D]))
            ident = singles.tile([P, P], f32, name="ident")
            make_identity(nc, ident)
            negc = singles.tile([P, 1], f32, name="negc")
            nc.vector.memset(negc, NEG)

            # edge-group stream loads (issued early, consumed after z)
            cb_sb = singles.tile([P, NG, SG, P], fp8, name="cb_sb")
            nb_sb = singles.tile([P, NG, SG, N], fp8, name="nb_sb")
            for r in range(NC_H):
                nc.sync.dma_start(
                    out=cb_sb[:, r],
                    in_=ch[r].rearrange("(s p) c -> p s c", p=P))
                nc.sync.dma_start(
                    out=nb_sb[:, r],
                    in_=nf[r].rearrange("(s p) c -> p s c", p=P))

            # z = nodes @ w (fp32); z_sb[p, c, d] holds z[c*128+p, d]
            z_sb = singles.tile([P, NC_N, D], f32, name="z_sb")
            u_sb = singles.tile([P, NC_H], f32, name="u_sb")
            wb_sb = singles.tile([P, NC_F], f32, name="wb_sb")
            pT_all = singles.tile([P, NC_H, N], f32, name="pT_all")
            with tc.tile_pool(name="z_ps", bufs=2, space="PSUM") as z_ps, \
                 tc.tile_pool(name="uscr", bufs=2) as uscr:
                for cn in range(NC_N):
                    zp = z_ps.tile([P, D], f32, name=f"zp_{cn}", tag="zp")
                    for cf in range(NC_F):
                        nc.tensor.matmul(
                            zp,
                            lhsT=nT_sb[:, cf, cn * P:(cn + 1) * P],
                            rhs=w_sb[:, cf, :],
                            start=(cf == 0), stop=(cf == NC_F - 1))
                    if cn % 2 == 0:
                        nc.vector.tensor_copy(z_sb[:, cn, :], zp)
                    else:
                        nc.scalar.copy(z_sb[:, cn, :], zp)
                    if cn < NC_H:
                        # u[n] = sum_d z[n,d] * a_top[d] (rows 0..511 only)
                        us = uscr.tile([P, D], f32, name=f"us_{cn}", tag="us")
                        nc.vector.tensor_mul(us, z_sb[:, cn, :], atop_b)
                        nc.vector.tensor_reduce(
                            u_sb[:, cn:cn + 1], us,
                            axis=mybir.AxisListType.X, op=Alu.add)
                # wb[f] = sum_d w[f,d] * a_bot[d]
                for cf in range(NC_F):
                    ws = uscr.tile([P, D], f32, name=f"ws_{cf}", tag="us")
                    nc.vector.tensor_mul(ws, w_sb[:, cf, :], abot_b)
                    nc.vector.tensor_reduce(
                        wb_sb[:, cf:cf + 1], ws,
                        axis=mybir.AxisListType.X, op=Alu.add)
                # v[m] = sum_f nodes[m,f] * wb[f] (PE matvec)
                v_row = singles.tile([1, N], f32, name="v_row")
                for jm in range(2):
                    vp = z_ps.tile([1, 512], f32, name=f"vp_{jm}", tag="zp")
                    for cf in range(NC_F):
                        nc.tensor.matmul(
                            vp,
                            lhsT=wb_sb[:, cf:cf + 1],
                            rhs=nT_sb[:, cf, jm * 512:(jm + 1) * 512],
                            start=(cf == 0), stop=(cf == NC_F - 1))
                    nc.vector.tensor_copy(
                        v_row[:, jm * 512:(jm + 1) * 512], vp)
                V_bc = singles.tile([P, N], f32, name="V_bc")
                nc.gpsimd.partition_broadcast(V_bc, v_row)

                # pT = leaky(u[n] + v[m]) per row chunk
                for r in range(NC_H):
                    t_uv = uscr.tile([P, N], f32, name=f"tuv_{r}", tag="tuv")
                    nc.scalar.activation(t_uv, V_bc, Act.Identity,
                                         bias=u_sb[:, r:r + 1], scale=1.0)
                    nc.vector.scalar_tensor_tensor(
                        out=pT_all[:, r, :], in0=t_uv, scalar=ALPHA,
                        in1=t_uv, op0=Alu.mult, op1=Alu.max)

            # ---- adjacency matmul + softmax + transpose + out, per r ----
            # adj row-chunk r only needs edge group r (grouped by source).
            # logits = adj*pT + (adj-1)*1e9
            # (exact: for adj==1 the +(adj-1)*1e9 term is exactly 0)
            eT_sb = singles.tile([P, NC_N, H], f32, name="eT_sb")
            out_r = out.rearrange("(r p) d -> p r d", p=P)
            with tc.tile_pool(name="adj_ps", bufs=2, space="PSUM") as adj_ps, \
                 tc.tile_pool(name="pscr", bufs=2) as pscr, \
                 tc.tile_pool(name="sml", bufs=4) as sml, \
                 tc.tile_pool(name="tp2_ps", bufs=2, space="PSUM") as tp2_ps, \
                 tc.tile_pool(name="o_ps", bufs=2, space="PSUM") as o_ps, \
                 tc.tile_pool(name="oscr", bufs=2) as oscr:
                for r in range(NC_H):
                    adj_pj = []
                    for j in range(2):
                        apj = adj_ps.tile([P, 512], f32,
                                          name=f"adj_{r}_{j}", tag=f"adj_{j}")
                        adj_pj.append(apj)
                        for t in range(SG // 2):
                            ks = slice(2 * t, 2 * t + 2)
                            nc.tensor.matmul(
                                apj,
                                lhsT=cb_sb[:, r, ks, :],
                                rhs=nb_sb[:, r, ks, j * 512:(j + 1) * 512],
                                start=(t == 0), stop=(t == SG // 2 - 1),
                                perf_mode=DR)
                    # softmax over m for this row chunk (reads PSUM directly)
                    m1 = pscr.tile([P, N], f32, name=f"m1_{r}", tag="m1")
                    for j in range(2):
                        nc.vector.tensor_mul(
                            m1[:, j * 512:(j + 1) * 512], adj_pj[j],
                            pT_all[:, r, j * 512:(j + 1) * 512])
                    s19 = pscr.tile([P, N], f32, name=f"s19_{r}", tag="s19")
                    for j in range(2):
                        # (adj-1)*1e9 via ACT free affine (fma: exact)
                        nc.scalar.activation(
                            s19[:, j * 512:(j + 1) * 512], adj_pj[j],
                            Act.Identity, bias=negc[:, 0:1], scale=-NEG)
                    lg = pscr.tile([P, N], f32, name=f"lg_{r}", tag="lg")
                    nc.vector.tensor_add(lg, m1, s19)
                    nmx = sml.tile([P, 1], f32, name=f"nmx_{r}", tag="nmx")
                    nc.vector.tensor_reduce(
                        nmx, lg, axis=mybir.AxisListType.X, op=Alu.max,
                        negate=True)
                    e_t = pscr.tile([P, N], f32, name=f"et_{r}", tag="et")
                    zr = sml.tile([P, 1], f32, name=f"zr_{r}", tag="zr")
                    nc.scalar.activation(e_t, lg, Act.Exp,
                                         bias=nmx[:, 0:1], scale=1.0,
                                         accum_out=zr)
                    ea = pscr.tile([P, N], f32, name=f"ea_{r}", tag="ea")
                    for j in range(2):
                        nc.vector.tensor_mul(
                            ea[:, j * 512:(j + 1) * 512],
                            e_t[:, j * 512:(j + 1) * 512], adj_pj[j])
                    rcp = sml.tile([P, 1], f32, name=f"rcp_{r}", tag="rcp")
                    nc.vector.reciprocal(rcp, zr)

                    # transpose expadj chunk into eT columns r*128..
                    for cm in range(NC_N):
                        pt2 = tp2_ps.tile([P, P], f32, name=f"tp2_{r}_{cm}",
                                          tag="tp2")
                        nc.tensor.transpose(
                            pt2, ea[:, cm * P:(cm + 1) * P], ident)
                        if (r + cm) % 2 == 0:
                            nc.vector.tensor_copy(
                                eT_sb[:, cm, r * P:(r + 1) * P], pt2)
                        else:
                            nc.scalar.copy(
                                eT_sb[:, cm, r * P:(r + 1) * P], pt2)

                    # out chunk = leaky(rcp * (expadj^T)^T @ z)
                    op = o_ps.tile([P, D], f32, name=f"op_{r}", tag="op")
                    for cm in range(NC_N):
                        nc.tensor.matmul(
                            op,
                            lhsT=eT_sb[:, cm, r * P:(r + 1) * P],
                            rhs=z_sb[:, cm, :],
                            start=(cm == 0), stop=(cm == NC_N - 1))
                    o_t = oscr.tile([P, D], f32, name=f"ot_{r}", tag="ot")
                    nc.vector.tensor_scalar(
                        out=o_t, in0=op, scalar1=rcp[:, 0:1],
                        scalar2=None, op0=Alu.mult)
                    o_l = oscr.tile([P, D], f32, name=f"ol_{r}", tag="ol")
                    nc.vector.scalar_tensor_tensor(
                        out=o_l, in0=o_t, scalar=ALPHA, in1=o_t,
                        op0=Alu.mult, op1=Alu.max)
                    nc.sync.dma_start(out=out_r[:, r, :], in_=o_l)

    nc.compile()
    return nc


def _get_compiled():
    global _compiled
    if _compiled is None:
        _compiled = _build()
    return _compiled


def _in_maps(nodes, Cmat, Nmat, w, attention):
    import ml_dtypes
    f8 = ml_dtypes.float8_e4m3
    nodes = np.asarray(nodes, dtype=np.float32)
    Cmat = np.asarray(Cmat, dtype=np.float32)
    Nmat = np.asarray(Nmat, dtype=np.float32)
    w = np.ascontiguousarray(np.asarray(w, dtype=np.float32))
    attention = np.asarray(attention, dtype=np.float32)
    atop = np.ascontiguousarray(attention[:D, 0][None, :])
    abot = np.ascontiguousarray(attention[D:, 0][None, :])
    maps = []
    for core in range(N_CORES):
        b, h = divmod(core, 2)
        lo, hi = h * H, (h + 1) * H
        src = Cmat[b].argmax(axis=1)
        ch_g = np.zeros((NG, EPG, P), dtype=f8)
        nf_g = np.zeros((NG, EPG, N), dtype=f8)
        for r in range(NG):
            glo = lo + r * P
            sel = np.nonzero((src >= glo) & (src < glo + P))[0]
            assert len(sel) <= EPG, f"group overflow: {len(sel)} > {EPG}"
            ch_g[r, :len(sel)] = Cmat[b][sel][:, glo:glo + P].astype(f8)
            nf_b = Nmat[b][sel]
            if h == 0:
                nf_g[r, :len(sel)] = nf_b.astype(f8)
            else:
                nf_g[r, :len(sel), :H] = nf_b[:, lo:hi].astype(f8)
                nf_g[r, :len(sel), H:] = nf_b[:, :lo].astype(f8)
        if h == 0:
            nodesT = nodes[b].T
        else:
            nodesT = np.concatenate([nodes[b, lo:hi], nodes[b, :lo]],
                                    axis=0).T
        maps.append({
            "ch": ch_g,
            "nf": nf_g,
            "nodesT": np.ascontiguousarray(nodesT),
            "w": w,
            "atop": atop,
            "abot": abot,
        })
    return maps


def kernel(nodes, Cmat, Nmat, mask, w, attention, _trace=False, _tmpdir=None):
    from concourse.bass_utils import run_bass_kernel_spmd

    nc = _get_compiled()
    maps = _in_maps(nodes, Cmat, Nmat, w, attention)
    res = run_bass_kernel_spmd(nc, maps, list(range(N_CORES)),
                               trace=_trace, tmpdir=_tmpdir)
    full = np.empty((B, N, D), dtype=np.float32)
    for core in range(N_CORES):
        b, h = divmod(core, 2)
        full[b, h * H:(h + 1) * H, :] = res.results[core]["out"]
    if _trace:
        return full, res
    return full


if __name__ == "__main__":
    rng = np.random.default_rng(0)
    src = rng.integers(0, N, (B, E))
    dst = rng.integers(0, N, (B, E))
    Cm = np.eye(N, dtype=np.float32)[src]
    Nm = np.eye(N, dtype=np.float32)[dst]
    nodes = rng.standard_normal((B, N, F)).astype(np.float32)
    w = (rng.standard_normal((F, D)) * 0.05).astype(np.float32)
    att = (rng.standard_normal((2 * D, 1)) * 0.05).astype(np.float32)
    mask = np.ones((B, N, N), dtype=bool)
    got = kernel(nodes, Cm, Nm, mask, w, att)
    print("kernel ran, output shape", got.shape)


# revision 10
# speedup vs baseline: 2.1003x; 1.0741x over previous
"""Trainium2 Bass kernel for nn_AttGraphConvLayer.

Reference computation (per batch b):
    z   = nodes @ w                          [N, D]
    z1  = Cmat @ z ; z2 = Nmat @ z           [E, D] (one-hot gathers)
    att = leaky_relu(concat(z1, z2) @ attention)      [E, 1]
    scores = (Cmat^T * att^T) @ Nmat         [N, N]
    adj    = Cmat^T @ Nmat                   [N, N]
    logits = scores + (1 - adj) * (-1e9)
    out = leaky_relu(softmax(logits, -1) * adj @ z)   [N, D]

Key identities used (Cmat/Nmat are one-hot incidence matrices):
  * att_e = leaky(u[src_e] + v[dst_e]) with u = z @ a_top, v = z @ a_bot.
    Hence scores[n, m] = adj[n, m] * leaky(u[n] + v[m]) -- no [E,D]
    gathers and no scores matmul are needed at all; only the adjacency
    matmul (contraction over E) remains.
  * adj has 0/1 inputs, so the adjacency matmul is EXACT in fp8 (e4m3;
    products are 0/1, fp32 PSUM accumulation) and runs ~8x faster than
    fp32 on the PE with perf_mode=DoubleRow (2 edges contracted per
    cell per cycle). The incidence matrices are shipped as fp8 from the
    host (exact, less DMA, no on-device casts).
  * v = z @ a_bot = nodes @ (w @ a_bot): computed via a tiny on-device
    reduction (wb = sum_d w*a_bot) plus a PE matvec against nodes^T.
    nodes^T itself is shipped from the host (layout choice), removing
    all PE transposes from the prologue.

Sharding: 8 cores = 4 batches x 2 row-halves (graph partitioning by
source node). A core's output rows n in [h*512,(h+1)*512) only receive
contributions from edges with src in that range, so the host ships each
core only those ~4096 edges, further grouped by 128-row source chunk
(each group padded with all-zero rows to a fixed 1280). Grouping makes
the one-hot source block only 128 columns wide, so each adjacency PSUM
tile needs just its own group's edges: 40 DoubleRow matmuls total.
All cores run the same program; the host permutes the node axis per core
so the core's 512 output rows are always rows 0..511 (applied
consistently to nodes rows, Cmat columns and Nmat columns; softmax and
the final contraction over the m axis are permutation invariant).
"""

import sys

for _p in ("/opt/trn_rl_repo", "/root/.axon_site/_ro/trn_rl_repo"):
    if _p not in sys.path:
        sys.path.insert(0, _p)

import numpy as np

B, E, N, F, D = 4, 8192, 1024, 512, 512
H = N // 2          # rows per core
P = 128
EPG = 1280          # padded edges per source-chunk group; group size is
                    # Binom(8192, 1/8): mean 1024, sd 30 -> 1280 is +8.5 sd
                    # (asserted at runtime)
NG = H // P         # 4 groups per core
ALPHA = 0.2
NEG = -1.0e9
N_CORES = 8

_compiled = None


def _build():
    import concourse.bacc as bacc
    import concourse.tile as tile
    import concourse.mybir as mybir
    from concourse.masks import make_identity

    dt = mybir.dt
    f32 = dt.float32
    fp8 = dt.float8e4
    Alu = mybir.AluOpType
    Act = mybir.ActivationFunctionType
    DR = mybir.MatmulPerfMode.DoubleRow

    nc = bacc.Bacc("TRN2", target_bir_lowering=False, debug=False,
                   num_devices=N_CORES)

    # edge groups: group r covers source rows r*128..(r+1)*128
    ch = nc.dram_tensor("ch", [NG, EPG, P], fp8, kind="ExternalInput").ap()
    nf = nc.dram_tensor("nf", [NG, EPG, N], fp8, kind="ExternalInput").ap()
    nodesT = nc.dram_tensor("nodesT", [F, N], f32, kind="ExternalInput").ap()
    w = nc.dram_tensor("w", [F, D], f32, kind="ExternalInput").ap()
    atop = nc.dram_tensor("atop", [1, D], f32, kind="ExternalInput").ap()
    abot = nc.dram_tensor("abot", [1, D], f32, kind="ExternalInput").ap()
    out = nc.dram_tensor("out", [H, D], f32, kind="ExternalOutput").ap()

    NC_N = N // P   # 8 node chunks
    NC_F = F // P   # 4 feature chunks
    NC_H = H // P   # 4 row chunks per core
    SG = EPG // P   # 10 sub-chunks of 128 edges per group
    nT_r = nodesT.rearrange("(c p) n -> p c n", p=P)
    w_r = w.rearrange("(c p) d -> p c d", p=P)

    with tile.TileContext(nc) as tc:
        with tc.tile_pool(name="singles", bufs=1) as singles:
            # ---- input loads: z's operands first, chunk-interleaved ----
            nT_sb = singles.tile([P, NC_F, N], f32, name="nT_sb")
            w_sb = singles.tile([P, NC_F, D], f32, name="w_sb")
            for cf in range(NC_F):
                nc.sync.dma_start(out=nT_sb[:, cf, :], in_=nT_r[:, cf, :])
                nc.sync.dma_start(out=w_sb[:, cf, :], in_=w_r[:, cf, :])
            atop_b = singles.tile([P, D], f32, name="atop_b")
            nc.sync.dma_start(out=atop_b, in_=atop.to_broadcast([P, D]))
            abot_b = singles.tile([P, D], f32, name="abot_b")
            nc.sync.dma_start(out=abot_b, in_=abot.to_broadcast([P, D]))
            ident = singles.tile([P, P], f32, name="ident")
            make_identity(nc, ident)
            negc = singles.tile([P, 1], f32, name="negc")
            nc.vector.memset(negc, NEG)

            # edge-group stream loads (issued early, consumed after z)
            cb_sb = singles.tile([P, NG, SG, P], fp8, name="cb_sb")
            nb_sb = singles.tile([P, NG, SG, N], fp8, name="nb_sb")
            for r in range(NC_H):
                nc.sync.dma_start(
                    out=cb_sb[:, r],
                    in_=ch[r].rearrange("(s p) c -> p s c", p=P))
                nc.sync.dma_start(
                    out=nb_sb[:, r],
                    in_=nf[r].rearrange("(s p) c -> p s c", p=P))

            # ---- z = nodes @ w (fp32), contraction pass outermost so the
            # first matmul only needs the first nT/w chunks ----
            z_sb = singles.tile([P, NC_N, D], f32, name="z_sb")
            u_sb = singles.tile([P, NC_H], f32, name="u_sb")
            wb_sb = singles.tile([P, NC_F], f32, name="wb_sb")
            pT_all = singles.tile([P, NC_H, N], f32, name="pT_all")
            v_row = singles.tile([1, N], f32, name="v_row")
            V_bc = singles.tile([P, N], f32, name="V_bc")
            with tc.tile_pool(name="z_ps", bufs=1, space="PSUM") as z_ps, \
                 tc.tile_pool(name="uscr", bufs=2) as uscr:
                zp = [z_ps.tile([P, D], f32, name=f"zp_{cn}", tag=f"zp_{cn}")
                      for cn in range(NC_N)]
                for cf in range(NC_F):
                    for cn in range(NC_N):
                        nc.tensor.matmul(
                            zp[cn],
                            lhsT=nT_sb[:, cf, cn * P:(cn + 1) * P],
                            rhs=w_sb[:, cf, :],
                            start=(cf == 0), stop=(cf == NC_F - 1))
                # wb[f] = sum_d w[f,d] * a_bot[d] (overlaps z matmuls)
                for cf in range(NC_F):
                    ws = uscr.tile([P, D], f32, name=f"ws_{cf}", tag="us")
                    nc.vector.tensor_mul(ws, w_sb[:, cf, :], abot_b)
                    nc.vector.tensor_reduce(
                        wb_sb[:, cf:cf + 1], ws,
                        axis=mybir.AxisListType.X, op=Alu.add)
                for cn in range(NC_N):
                    if cn % 2 == 0:
                        nc.vector.tensor_copy(z_sb[:, cn, :], zp[cn])
                    else:
                        nc.scalar.copy(z_sb[:, cn, :], zp[cn])
                    if cn < NC_H:
                        # u[n] = sum_d z[n,d] * a_top[d] (rows 0..511 only)
                        us = uscr.tile([P, D], f32, name=f"us_{cn}", tag="us")
                        nc.vector.tensor_mul(us, z_sb[:, cn, :], atop_b)
                        nc.vector.tensor_reduce(
                            u_sb[:, cn:cn + 1], us,
                            axis=mybir.AxisListType.X, op=Alu.add)
                # v[m] = sum_f nodes[m,f] * wb[f]; reuses zp_0/zp_1 slots
                for jm in range(2):
                    vp = z_ps.tile([1, 512], f32, name=f"vp_{jm}",
                                   tag=f"zp_{jm}")
                    for cf in range(NC_F):
                        nc.tensor.matmul(
                            vp,
                            lhsT=wb_sb[:, cf:cf + 1],
                            rhs=nT_sb[:, cf, jm * 512:(jm + 1) * 512],
                            start=(cf == 0), stop=(cf == NC_F - 1))
                    nc.vector.tensor_copy(
                        v_row[:, jm * 512:(jm + 1) * 512], vp)
                nc.gpsimd.partition_broadcast(V_bc, v_row)

                # pT = leaky(u[n] + v[m]) per row chunk
                for r in range(NC_H):
                    t_uv = uscr.tile([P, N], f32, name=f"tuv_{r}", tag="tuv")
                    nc.scalar.activation(t_uv, V_bc, Act.Identity,
                                         bias=u_sb[:, r:r + 1], scale=1.0)
                    nc.vector.scalar_tensor_tensor(
                        out=pT_all[:, r, :], in0=t_uv, scalar=ALPHA,
                        in1=t_uv, op0=Alu.mult, op1=Alu.max)

            # ---- adjacency matmul + softmax + transpose + out, per r ----
            # adj row-chunk r only needs edge group r (grouped by source).
            # logits = adj*pT + (adj-1)*1e9
            # (exact: for adj==1 the +(adj-1)*1e9 term is exactly 0)
            # softmax pipeline runs in m-halves (j = 0/1) to shorten the
            # serial chain; adjacency stays resident in PSUM.
            eT_sb = singles.tile([P, NC_N, H], f32, name="eT_sb")
            out_r = out.rearrange("(r p) d -> p r d", p=P)
            with tc.tile_pool(name="adj_ps", bufs=2, space="PSUM") as adj_ps, \
                 tc.tile_pool(name="pscr", bufs=2) as pscr, \
                 tc.tile_pool(name="sml", bufs=6) as sml, \
                 tc.tile_pool(name="tp2_ps", bufs=2, space="PSUM") as tp2_ps, \
                 tc.tile_pool(name="o_ps", bufs=2, space="PSUM") as o_ps, \
                 tc.tile_pool(name="oscr", bufs=2) as oscr:
                for r in range(NC_H):
                    adj_pj = []
                    for j in range(2):
                        apj = adj_ps.tile([P, 512], f32,
                                          name=f"adj_{r}_{j}", tag=f"adj_{j}")
                        adj_pj.append(apj)
                        for t in range(SG // 2):
                            ks = slice(2 * t, 2 * t + 2)
                            nc.tensor.matmul(
                                apj,
                                lhsT=cb_sb[:, r, ks, :],
                                rhs=nb_sb[:, r, ks, j * 512:(j + 1) * 512],
                                start=(t == 0), stop=(t == SG // 2 - 1),
                                perf_mode=DR)
                    # softmax over m, pipelined in halves (reads PSUM)
                    m1 = pscr.tile([P, N], f32, name=f"m1_{r}", tag="m1")
                    s19 = pscr.tile([P, N], f32, name=f"s19_{r}", tag="s19")
                    lg = pscr.tile([P, N], f32, name=f"lg_{r}", tag="lg")
                    nmh = sml.tile([P, 2], f32, name=f"nmh_{r}", tag="nmh")
                    for j in range(2):
                        sl = slice(j * 512, (j + 1) * 512)
                        nc.vector.tensor_mul(m1[:, sl], adj_pj[j],
                                             pT_all[:, r, sl])
                        # (adj-1)*1e9 via ACT free affine (fma: exact)
                        nc.scalar.activation(s19[:, sl], adj_pj[j],
                                             Act.Identity,
                                             bias=negc[:, 0:1], scale=-NEG)
                        nc.vector.tensor_add(lg[:, sl], m1[:, sl], s19[:, sl])
                        nc.vector.tensor_reduce(
                            nmh[:, j:j + 1], lg[:, sl],
                            axis=mybir.AxisListType.X, op=Alu.max,
                            negate=True)
                    nmx = sml.tile([P, 1], f32, name=f"nmx_{r}", tag="nmx")
                    # -max(a,b) = min(-a,-b)
                    nc.vector.tensor_tensor(nmx, nmh[:, 0:1], nmh[:, 1:2],
                                            op=Alu.min)
                    e_t = pscr.tile([P, N], f32, name=f"et_{r}", tag="et")
                    ea = pscr.tile([P, N], f32, name=f"ea_{r}", tag="ea")
                    zh = sml.tile([P, 2], f32, name=f"zh_{r}", tag="zh")
                    for j in range(2):
                        sl = slice(j * 512, (j + 1) * 512)
                        nc.scalar.activation(e_t[:, sl], lg[:, sl], Act.Exp,
                                             bias=nmx[:, 0:1], scale=1.0,
                                             accum_out=zh[:, j:j + 1])
                        nc.vector.tensor_mul(ea[:, sl], e_t[:, sl], adj_pj[j])
                    zr = sml.tile([P, 1], f32, name=f"zr_{r}", tag="zr")
                    nc.vector.tensor_add(zr, zh[:, 0:1], zh[:, 1:2])
                    rcp = sml.tile([P, 1], f32, name=f"rcp_{r}", tag="rcp")
                    nc.vector.reciprocal(rcp, zr)

                    # transpose expadj chunk into eT columns r*128..
                    for cm in range(NC_N):
                        pt2 = tp2_ps.tile([P, P], f32, name=f"tp2_{r}_{cm}",
                                          tag="tp2")
                        nc.tensor.transpose(
                            pt2, ea[:, cm * P:(cm + 1) * P], ident)
                        if cm % 4 == 0:
                            nc.vector.tensor_copy(
                                eT_sb[:, cm, r * P:(r + 1) * P], pt2)
                        else:
                            nc.scalar.copy(
                                eT_sb[:, cm, r * P:(r + 1) * P], pt2)

                    # out chunk = leaky(rcp * (expadj^T)^T @ z)
                    op = o_ps.tile([P, D], f32, name=f"op_{r}", tag="op")
                    for cm in range(NC_N):
                        nc.tensor.matmul(
                            op,
                            lhsT=eT_sb[:, cm, r * P:(r + 1) * P],
                            rhs=z_sb[:, cm, :],
                            start=(cm == 0), stop=(cm == NC_N - 1))
                    o_t = oscr.tile([P, D], f32, name=f"ot_{r}", tag="ot")
                    nc.scalar.mul(o_t, op, rcp[:, 0:1])
                    o_l = oscr.tile([P, D], f32, name=f"ol_{r}", tag="ol")
                    nc.vector.scalar_tensor_tensor(
                        out=o_l, in0=o_t, scalar=ALPHA, in1=o_t,
                        op0=Alu.mult, op1=Alu.max)
                    nc.sync.dma_start(out=out_r[:, r, :], in_=o_l)

    nc.compile()
    return nc


def _get_compiled():
    global _compiled
    if _compiled is None:
        _compiled = _build()
    return _compiled


def _in_maps(nodes, Cmat, Nmat, w, attention):
    import ml_dtypes
    f8 = ml_dtypes.float8_e4m3
    nodes = np.asarray(nodes, dtype=np.float32)
    Cmat = np.asarray(Cmat, dtype=np.float32)
    Nmat = np.asarray(Nmat, dtype=np.float32)
    w = np.ascontiguousarray(np.asarray(w, dtype=np.float32))
    attention = np.asarray(attention, dtype=np.float32)
    atop = np.ascontiguousarray(attention[:D, 0][None, :])
    abot = np.ascontiguousarray(attention[D:, 0][None, :])
    maps = []
    for core in range(N_CORES):
        b, h = divmod(core, 2)
        lo, hi = h * H, (h + 1) * H
        src = Cmat[b].argmax(axis=1)
        ch_g = np.zeros((NG, EPG, P), dtype=f8)
        nf_g = np.zeros((NG, EPG, N), dtype=f8)
        for r in range(NG):
            glo = lo + r * P
            sel = np.nonzero((src >= glo) & (src < glo + P))[0]
            assert len(sel) <= EPG, f"group overflow: {len(sel)} > {EPG}"
            ch_g[r, :len(sel)] = Cmat[b][sel][:, glo:glo + P].astype(f8)
            nf_b = Nmat[b][sel]
            if h == 0:
                nf_g[r, :len(sel)] = nf_b.astype(f8)
            else:
                nf_g[r, :len(sel), :H] = nf_b[:, lo:hi].astype(f8)
                nf_g[r, :len(sel), H:] = nf_b[:, :lo].astype(f8)
        if h == 0:
            nodesT = nodes[b].T
        else:
            nodesT = np.concatenate([nodes[b, lo:hi], nodes[b, :lo]],
                                    axis=0).T
        maps.append({
            "ch": ch_g,
            "nf": nf_g,
            "nodesT": np.ascontiguousarray(nodesT),
            "w": w,
            "atop": atop,
            "abot": abot,
        })
    return maps


def kernel(nodes, Cmat, Nmat, mask, w, attention, _trace=False, _tmpdir=None):
    from concourse.bass_utils import run_bass_kernel_spmd

    nc = _get_compiled()
    maps = _in_maps(nodes, Cmat, Nmat, w, attention)
    res = run_bass_kernel_spmd(nc, maps, list(range(N_CORES)),
                               trace=_trace, tmpdir=_tmpdir)
    full = np.empty((B, N, D), dtype=np.float32)
    for core in range(N_CORES):
        b, h = divmod(core, 2)
        full[b, h * H:(h + 1) * H, :] = res.results[core]["out"]
    if _trace:
        return full, res
    return full


if __name__ == "__main__":
    rng = np.random.default_rng(0)
    src = rng.integers(0, N, (B, E))
    dst = rng.integers(0, N, (B, E))
    Cm = np.eye(N, dtype=np.float32)[src]
    Nm = np.eye(N, dtype=np.float32)[dst]
    nodes = rng.standard_normal((B, N, F)).astype(np.float32)
    w = (rng.standard_normal((F, D)) * 0.05).astype(np.float32)
    att = (rng.standard_normal((2 * D, 1)) * 0.05).astype(np.float32)
    mask = np.ones((B, N, N), dtype=bool)
    got = kernel(nodes, Cm, Nm, mask, w, att)
    print("kernel ran, output shape", got.shape)


# revision 11
# speedup vs baseline: 2.2151x; 1.0547x over previous
"""Trainium2 Bass kernel for nn_AttGraphConvLayer.

Reference computation (per batch b):
    z   = nodes @ w                          [N, D]
    z1  = Cmat @ z ; z2 = Nmat @ z           [E, D] (one-hot gathers)
    att = leaky_relu(concat(z1, z2) @ attention)      [E, 1]
    scores = (Cmat^T * att^T) @ Nmat         [N, N]
    adj    = Cmat^T @ Nmat                   [N, N]
    logits = scores + (1 - adj) * (-1e9)
    out = leaky_relu(softmax(logits, -1) * adj @ z)   [N, D]

Key identities used (Cmat/Nmat are one-hot incidence matrices):
  * att_e = leaky(u[src_e] + v[dst_e]) with u = z @ a_top, v = z @ a_bot.
    Hence scores[n, m] = adj[n, m] * leaky(u[n] + v[m]) -- no [E,D]
    gathers and no scores matmul are needed at all; only the adjacency
    matmul (contraction over E) remains.
  * adj has 0/1 inputs, so the adjacency matmul is EXACT in fp8 (e4m3;
    products are 0/1, fp32 PSUM accumulation) and runs ~8x faster than
    fp32 on the PE with perf_mode=DoubleRow (2 edges contracted per
    cell per cycle). The incidence matrices are shipped as fp8 from the
    host (exact, less DMA, no on-device casts).
  * v = z @ a_bot = nodes @ (w @ a_bot): computed via a tiny on-device
    reduction (wb = sum_d w*a_bot) plus a PE matvec against nodes^T.
    nodes^T itself is shipped from the host (layout choice), removing
    all PE transposes from the prologue.

Sharding: 8 cores = 4 batches x 2 row-halves (graph partitioning by
source node). A core's output rows n in [h*512,(h+1)*512) only receive
contributions from edges with src in that range, so the host ships each
core only those ~4096 edges, further grouped by 128-row source chunk
(each group padded with all-zero rows to a fixed 1280). Grouping makes
the one-hot source block only 128 columns wide, so each adjacency PSUM
tile needs just its own group's edges: 40 DoubleRow matmuls total.
All cores run the same program; the host permutes the node axis per core
so the core's 512 output rows are always rows 0..511 (applied
consistently to nodes rows, Cmat columns and Nmat columns; softmax and
the final contraction over the m axis are permutation invariant).
"""

import sys

for _p in ("/opt/trn_rl_repo", "/root/.axon_site/_ro/trn_rl_repo"):
    if _p not in sys.path:
        sys.path.insert(0, _p)

import numpy as np

B, E, N, F, D = 4, 8192, 1024, 512, 512
H = N // 2          # rows per core
P = 128
EPG = 1280          # padded edges per source-chunk group; group size is
                    # Binom(8192, 1/8): mean 1024, sd 30 -> 1280 is +8.5 sd
                    # (asserted at runtime)
NG = H // P         # 4 groups per core
ALPHA = 0.2
NEG = -1.0e9
N_CORES = 8

_compiled = None


def _build():
    import concourse.bacc as bacc
    import concourse.tile as tile
    import concourse.mybir as mybir
    from concourse.masks import make_identity

    dt = mybir.dt
    f32 = dt.float32
    fp8 = dt.float8e4
    Alu = mybir.AluOpType
    Act = mybir.ActivationFunctionType
    DR = mybir.MatmulPerfMode.DoubleRow

    nc = bacc.Bacc("TRN2", target_bir_lowering=False, debug=False,
                   num_devices=N_CORES)

    # edge groups: group r covers source rows r*128..(r+1)*128
    ch = nc.dram_tensor("ch", [NG, EPG, P], fp8, kind="ExternalInput").ap()
    nf = nc.dram_tensor("nf", [NG, EPG, N], fp8, kind="ExternalInput").ap()
    nodesT = nc.dram_tensor("nodesT", [F, N], f32, kind="ExternalInput").ap()
    w = nc.dram_tensor("w", [F, D], f32, kind="ExternalInput").ap()
    atop = nc.dram_tensor("atop", [1, D], f32, kind="ExternalInput").ap()
    abot = nc.dram_tensor("abot", [1, D], f32, kind="ExternalInput").ap()
    out = nc.dram_tensor("out", [H, D], f32, kind="ExternalOutput").ap()

    NC_N = N // P   # 8 node chunks
    NC_F = F // P   # 4 feature chunks
    NC_H = H // P   # 4 row chunks per core
    SG = EPG // P   # 10 sub-chunks of 128 edges per group
    nT_r = nodesT.rearrange("(c p) n -> p c n", p=P)
    w_r = w.rearrange("(c p) d -> p c d", p=P)

    with tile.TileContext(nc) as tc:
        with tc.tile_pool(name="singles", bufs=1) as singles:
            # ---- input loads: z's operands first, chunk-interleaved ----
            nT_sb = singles.tile([P, NC_F, N], f32, name="nT_sb")
            w_sb = singles.tile([P, NC_F, D], f32, name="w_sb")
            for cf in range(NC_F):
                nc.sync.dma_start(out=nT_sb[:, cf, :], in_=nT_r[:, cf, :])
                nc.sync.dma_start(out=w_sb[:, cf, :], in_=w_r[:, cf, :])
            atop_b = singles.tile([P, D], f32, name="atop_b")
            nc.sync.dma_start(out=atop_b, in_=atop.to_broadcast([P, D]))
            abot_b = singles.tile([P, D], f32, name="abot_b")
            nc.sync.dma_start(out=abot_b, in_=abot.to_broadcast([P, D]))
            ident = singles.tile([P, P], f32, name="ident")
            make_identity(nc, ident)
            negc = singles.tile([P, 1], f32, name="negc")
            nc.vector.memset(negc, NEG)

            # edge-group stream loads (issued early, consumed after z)
            cb_sb = singles.tile([P, NG, SG, P], fp8, name="cb_sb")
            nb_sb = singles.tile([P, NG, SG, N], fp8, name="nb_sb")
            for r in range(NC_H):
                nc.sync.dma_start(
                    out=cb_sb[:, r],
                    in_=ch[r].rearrange("(s p) c -> p s c", p=P))
                nc.sync.dma_start(
                    out=nb_sb[:, r],
                    in_=nf[r].rearrange("(s p) c -> p s c", p=P))

            # ---- z = nodes @ w (fp32), contraction pass outermost so the
            # first matmul only needs the first nT/w chunks ----
            z_sb = singles.tile([P, NC_N, D], f32, name="z_sb")
            u_sb = singles.tile([P, NC_H], f32, name="u_sb")
            wb_sb = singles.tile([P, NC_F], f32, name="wb_sb")
            pT_all = singles.tile([P, NC_H, N], f32, name="pT_all")
            v_row = singles.tile([1, N], f32, name="v_row")
            V_bc = singles.tile([P, N], f32, name="V_bc")
            with tc.tile_pool(name="z_ps", bufs=1, space="PSUM") as z_ps, \
                 tc.tile_pool(name="uscr", bufs=2) as uscr:
                # wb[f] = sum_d w[f,d] * a_bot[d]
                for cf in range(NC_F):
                    ws = uscr.tile([P, D], f32, name=f"ws_{cf}", tag="us")
                    nc.vector.tensor_mul(ws, w_sb[:, cf, :], abot_b)
                    nc.vector.tensor_reduce(
                        wb_sb[:, cf:cf + 1], ws,
                        axis=mybir.AxisListType.X, op=Alu.add)
                # v[m] = sum_f nodes[m,f] * wb[f] -- before z so pT can
                # start as soon as the first z rows are done
                for jm in range(2):
                    vp = z_ps.tile([1, 512], f32, name=f"vp_{jm}",
                                   tag=f"zp_{4 + jm}")
                    for cf in range(NC_F):
                        nc.tensor.matmul(
                            vp,
                            lhsT=wb_sb[:, cf:cf + 1],
                            rhs=nT_sb[:, cf, jm * 512:(jm + 1) * 512],
                            start=(cf == 0), stop=(cf == NC_F - 1))
                    nc.vector.tensor_copy(
                        v_row[:, jm * 512:(jm + 1) * 512], vp)
                nc.gpsimd.partition_broadcast(V_bc, v_row)

                zp = [z_ps.tile([P, D], f32, name=f"zp_{cn}", tag=f"zp_{cn}")
                      for cn in range(NC_N)]
                # z rows 0..511 first: u and pT for the row chunks overlap
                # the second half of the z matmul
                for half in (0, 1):
                    cns = range(4 * half, 4 * half + 4)
                    for cf in range(NC_F):
                        for cn in cns:
                            nc.tensor.matmul(
                                zp[cn],
                                lhsT=nT_sb[:, cf, cn * P:(cn + 1) * P],
                                rhs=w_sb[:, cf, :],
                                start=(cf == 0), stop=(cf == NC_F - 1))
                    for cn in cns:
                        if cn % 2 == 0:
                            nc.vector.tensor_copy(z_sb[:, cn, :], zp[cn])
                        else:
                            nc.scalar.copy(z_sb[:, cn, :], zp[cn])
                        if cn < NC_H:
                            # u[n] = sum_d z[n,d] * a_top[d]
                            us = uscr.tile([P, D], f32, name=f"us_{cn}",
                                           tag="us")
                            nc.vector.tensor_mul(us, z_sb[:, cn, :], atop_b)
                            nc.vector.tensor_reduce(
                                u_sb[:, cn:cn + 1], us,
                                axis=mybir.AxisListType.X, op=Alu.add)
                            # pT = leaky(u[n] + v[m]) for this row chunk
                            r = cn
                            t_uv = uscr.tile([P, N], f32, name=f"tuv_{r}",
                                             tag="tuv")
                            nc.scalar.activation(t_uv, V_bc, Act.Identity,
                                                 bias=u_sb[:, r:r + 1],
                                                 scale=1.0)
                            nc.vector.scalar_tensor_tensor(
                                out=pT_all[:, r, :], in0=t_uv, scalar=ALPHA,
                                in1=t_uv, op0=Alu.mult, op1=Alu.max)

            # ---- adjacency matmul + softmax + transpose + out, per r ----
            # adj row-chunk r only needs edge group r (grouped by source).
            # logits = adj*pT + (adj-1)*1e9
            # (exact: for adj==1 the +(adj-1)*1e9 term is exactly 0)
            # softmax pipeline runs in m-halves (j = 0/1) to shorten the
            # serial chain; adjacency stays resident in PSUM.
            eT_sb = singles.tile([P, NC_N, H], f32, name="eT_sb")
            out_r = out.rearrange("(r p) d -> p r d", p=P)
            with tc.tile_pool(name="adj_ps", bufs=2, space="PSUM") as adj_ps, \
                 tc.tile_pool(name="pscr", bufs=2) as pscr, \
                 tc.tile_pool(name="sml", bufs=6) as sml, \
                 tc.tile_pool(name="tp2_ps", bufs=2, space="PSUM") as tp2_ps, \
                 tc.tile_pool(name="o_ps", bufs=2, space="PSUM") as o_ps, \
                 tc.tile_pool(name="oscr", bufs=2) as oscr:
                for r in range(NC_H):
                    adj_pj = []
                    for j in range(2):
                        apj = adj_ps.tile([P, 512], f32,
                                          name=f"adj_{r}_{j}", tag=f"adj_{j}")
                        adj_pj.append(apj)
                        for t in range(SG // 2):
                            ks = slice(2 * t, 2 * t + 2)
                            nc.tensor.matmul(
                                apj,
                                lhsT=cb_sb[:, r, ks, :],
                                rhs=nb_sb[:, r, ks, j * 512:(j + 1) * 512],
                                start=(t == 0), stop=(t == SG // 2 - 1),
                                perf_mode=DR)
                    # softmax over m, pipelined in halves (reads PSUM)
                    m1 = pscr.tile([P, N], f32, name=f"m1_{r}", tag="m1")
                    s19 = pscr.tile([P, N], f32, name=f"s19_{r}", tag="s19")
                    lg = pscr.tile([P, N], f32, name=f"lg_{r}", tag="lg")
                    nmh = sml.tile([P, 2], f32, name=f"nmh_{r}", tag="nmh")
                    for j in range(2):
                        sl = slice(j * 512, (j + 1) * 512)
                        nc.vector.tensor_mul(m1[:, sl], adj_pj[j],
                                             pT_all[:, r, sl])
                        # (adj-1)*1e9 via ACT free affine (fma: exact)
                        nc.scalar.activation(s19[:, sl], adj_pj[j],
                                             Act.Identity,
                                             bias=negc[:, 0:1], scale=-NEG)
                        nc.vector.tensor_add(lg[:, sl], m1[:, sl], s19[:, sl])
                        nc.vector.tensor_reduce(
                            nmh[:, j:j + 1], lg[:, sl],
                            axis=mybir.AxisListType.X, op=Alu.max,
                            negate=True)
                    nmx = sml.tile([P, 1], f32, name=f"nmx_{r}", tag="nmx")
                    # -max(a,b) = min(-a,-b)
                    nc.vector.tensor_tensor(nmx, nmh[:, 0:1], nmh[:, 1:2],
                                            op=Alu.min)
                    e_t = pscr.tile([P, N], f32, name=f"et_{r}", tag="et")
                    ea = pscr.tile([P, N], f32, name=f"ea_{r}", tag="ea")
                    zh = sml.tile([P, 2], f32, name=f"zh_{r}", tag="zh")
                    for j in range(2):
                        sl = slice(j * 512, (j + 1) * 512)
                        nc.scalar.activation(e_t[:, sl], lg[:, sl], Act.Exp,
                                             bias=nmx[:, 0:1], scale=1.0,
                                             accum_out=zh[:, j:j + 1])
                        nc.vector.tensor_mul(ea[:, sl], e_t[:, sl], adj_pj[j])
                    zr = sml.tile([P, 1], f32, name=f"zr_{r}", tag="zr")
                    nc.vector.tensor_add(zr, zh[:, 0:1], zh[:, 1:2])
                    rcp = sml.tile([P, 1], f32, name=f"rcp_{r}", tag="rcp")
                    nc.vector.reciprocal(rcp, zr)

                    # transpose expadj chunk into eT columns r*128..
                    for cm in range(NC_N):
                        pt2 = tp2_ps.tile([P, P], f32, name=f"tp2_{r}_{cm}",
                                          tag="tp2")
                        nc.tensor.transpose(
                            pt2, ea[:, cm * P:(cm + 1) * P], ident)
                        if cm % 4 == 0:
                            nc.vector.tensor_copy(
                                eT_sb[:, cm, r * P:(r + 1) * P], pt2)
                        else:
                            nc.scalar.copy(
                                eT_sb[:, cm, r * P:(r + 1) * P], pt2)

                    # out chunk = leaky(rcp * (expadj^T)^T @ z)
                    op = o_ps.tile([P, D], f32, name=f"op_{r}", tag="op")
                    for cm in range(NC_N):
                        nc.tensor.matmul(
                            op,
                            lhsT=eT_sb[:, cm, r * P:(r + 1) * P],
                            rhs=z_sb[:, cm, :],
                            start=(cm == 0), stop=(cm == NC_N - 1))
                    o_t = oscr.tile([P, D], f32, name=f"ot_{r}", tag="ot")
                    nc.scalar.mul(o_t, op, rcp[:, 0:1])
                    o_l = oscr.tile([P, D], f32, name=f"ol_{r}", tag="ol")
                    nc.vector.scalar_tensor_tensor(
                        out=o_l, in0=o_t, scalar=ALPHA, in1=o_t,
                        op0=Alu.mult, op1=Alu.max)
                    nc.sync.dma_start(out=out_r[:, r, :], in_=o_l)

    nc.compile()
    return nc


def _get_compiled():
    global _compiled
    if _compiled is None:
        _compiled = _build()
    return _compiled


def _in_maps(nodes, Cmat, Nmat, w, attention):
    import ml_dtypes
    f8 = ml_dtypes.float8_e4m3
    nodes = np.asarray(nodes, dtype=np.float32)
    Cmat = np.asarray(Cmat, dtype=np.float32)
    Nmat = np.asarray(Nmat, dtype=np.float32)
    w = np.ascontiguousarray(np.asarray(w, dtype=np.float32))
    attention = np.asarray(attention, dtype=np.float32)
    atop = np.ascontiguousarray(attention[:D, 0][None, :])
    abot = np.ascontiguousarray(attention[D:, 0][None, :])
    maps = []
    for core in range(N_CORES):
        b, h = divmod(core, 2)
        lo, hi = h * H, (h + 1) * H
        src = Cmat[b].argmax(axis=1)
        ch_g = np.zeros((NG, EPG, P), dtype=f8)
        nf_g = np.zeros((NG, EPG, N), dtype=f8)
        for r in range(NG):
            glo = lo + r * P
            sel = np.nonzero((src >= glo) & (src < glo + P))[0]
            assert len(sel) <= EPG, f"group overflow: {len(sel)} > {EPG}"
            ch_g[r, :len(sel)] = Cmat[b][sel][:, glo:glo + P].astype(f8)
            nf_b = Nmat[b][sel]
            if h == 0:
                nf_g[r, :len(sel)] = nf_b.astype(f8)
            else:
                nf_g[r, :len(sel), :H] = nf_b[:, lo:hi].astype(f8)
                nf_g[r, :len(sel), H:] = nf_b[:, :lo].astype(f8)
        if h == 0:
            nodesT = nodes[b].T
        else:
            nodesT = np.concatenate([nodes[b, lo:hi], nodes[b, :lo]],
                                    axis=0).T
        maps.append({
            "ch": ch_g,
            "nf": nf_g,
            "nodesT": np.ascontiguousarray(nodesT),
            "w": w,
            "atop": atop,
            "abot": abot,
        })
    return maps


def kernel(nodes, Cmat, Nmat, mask, w, attention, _trace=False, _tmpdir=None):
    from concourse.bass_utils import run_bass_kernel_spmd

    nc = _get_compiled()
    maps = _in_maps(nodes, Cmat, Nmat, w, attention)
    res = run_bass_kernel_spmd(nc, maps, list(range(N_CORES)),
                               trace=_trace, tmpdir=_tmpdir)
    full = np.empty((B, N, D), dtype=np.float32)
    for core in range(N_CORES):
        b, h = divmod(core, 2)
        full[b, h * H:(h + 1) * H, :] = res.results[core]["out"]
    if _trace:
        return full, res
    return full


if __name__ == "__main__":
    rng = np.random.default_rng(0)
    src = rng.integers(0, N, (B, E))
    dst = rng.integers(0, N, (B, E))
    Cm = np.eye(N, dtype=np.float32)[src]
    Nm = np.eye(N, dtype=np.float32)[dst]
    nodes = rng.standard_normal((B, N, F)).astype(np.float32)
    w = (rng.standard_normal((F, D)) * 0.05).astype(np.float32)
    att = (rng.standard_normal((2 * D, 1)) * 0.05).astype(np.float32)
    mask = np.ones((B, N, N), dtype=bool)
    got = kernel(nodes, Cm, Nm, mask, w, att)
    print("kernel ran, output shape", got.shape)


# revision 16
# speedup vs baseline: 2.7584x; 1.2453x over previous
"""Trainium2 Bass kernel for nn_AttGraphConvLayer.

Reference computation (per batch b):
    z   = nodes @ w                          [N, D]
    z1  = Cmat @ z ; z2 = Nmat @ z           [E, D] (one-hot gathers)
    att = leaky_relu(concat(z1, z2) @ attention)      [E, 1]
    scores = (Cmat^T * att^T) @ Nmat         [N, N]
    adj    = Cmat^T @ Nmat                   [N, N]
    logits = scores + (1 - adj) * (-1e9)
    out = leaky_relu(softmax(logits, -1) * adj @ z)   [N, D]

Key identities used (Cmat/Nmat are one-hot incidence matrices):
  * att_e = leaky(u[src_e] + v[dst_e]) with u = z @ a_top, v = z @ a_bot.
    Hence scores[n, m] = adj[n, m] * leaky(u[n] + v[m]) -- no [E,D]
    gathers and no scores matmul are needed at all; only the adjacency
    matmul (contraction over E) remains.
  * adj has 0/1 inputs, so the adjacency matmul is EXACT in fp8 (e4m3;
    products are 0/1, fp32 PSUM accumulation) and runs ~8x faster than
    fp32 on the PE with perf_mode=DoubleRow (2 edges contracted per
    cell per cycle). The incidence matrices are shipped as fp8 from the
    host (exact, less DMA, no on-device casts).
  * v = z @ a_bot = nodes @ (w @ a_bot): computed via a tiny on-device
    reduction (wb = sum_d w*a_bot) plus a PE matvec against nodes^T.
    nodes^T itself is shipped from the host (layout choice), removing
    all PE transposes from the prologue.

Sharding: 8 cores = 4 batches x 2 row-halves (graph partitioning by
source node). A core's output rows n in [h*512,(h+1)*512) only receive
contributions from edges with src in that range, so the host ships each
core only those ~4096 edges, further grouped by 128-row source chunk
(each group padded with all-zero rows to a fixed 1280). Grouping makes
the one-hot source block only 128 columns wide, so each adjacency PSUM
tile needs just its own group's edges: 40 DoubleRow matmuls total.
All cores run the same program; the host permutes the node axis per core
so the core's 512 output rows are always rows 0..511 (applied
consistently to nodes rows, Cmat columns and Nmat columns; softmax and
the final contraction over the m axis are permutation invariant).
"""

import sys

for _p in ("/opt/trn_rl_repo", "/root/.axon_site/_ro/trn_rl_repo"):
    if _p not in sys.path:
        sys.path.insert(0, _p)

import numpy as np

B, E, N, F, D = 4, 8192, 1024, 512, 512
H = N // 2          # rows per core
P = 128
EPG = 1280          # padded edges per source-chunk group; group size is
                    # Binom(8192, 1/8): mean 1024, sd 30 -> 1280 is +8.5 sd
                    # (asserted at runtime)
NG = H // P         # 4 groups per core
ALPHA = 0.2
NEG = -1.0e9
N_CORES = 8

_compiled = None


def _build():
    import concourse.bacc as bacc
    import concourse.tile as tile
    import concourse.mybir as mybir
    from concourse.masks import make_identity

    dt = mybir.dt
    f32 = dt.float32
    fp8 = dt.float8e4
    Alu = mybir.AluOpType
    Act = mybir.ActivationFunctionType
    DR = mybir.MatmulPerfMode.DoubleRow

    nc = bacc.Bacc("TRN2", target_bir_lowering=False, debug=False,
                   num_devices=N_CORES)

    # edge groups: group r covers source rows r*128..(r+1)*128
    ch = nc.dram_tensor("ch", [NG, EPG, P], fp8, kind="ExternalInput").ap()
    nf = nc.dram_tensor("nf", [NG, EPG, N], fp8, kind="ExternalInput").ap()
    nodesT = nc.dram_tensor("nodesT", [F, N], f32, kind="ExternalInput").ap()
    w = nc.dram_tensor("w", [F, D], f32, kind="ExternalInput").ap()
    atop = nc.dram_tensor("atop", [1, D], f32, kind="ExternalInput").ap()
    abot = nc.dram_tensor("abot", [1, D], f32, kind="ExternalInput").ap()
    out = nc.dram_tensor("out", [H, D], f32, kind="ExternalOutput").ap()

    NC_N = N // P   # 8 node chunks
    NC_F = F // P   # 4 feature chunks
    NC_H = H // P   # 4 row chunks per core
    SG = EPG // P   # 10 sub-chunks of 128 edges per group
    nT_r = nodesT.rearrange("(c p) n -> p c n", p=P)
    w_r = w.rearrange("(c p) d -> p c d", p=P)

    with tile.TileContext(nc) as tc:
        with tc.tile_pool(name="singles", bufs=1) as singles:
            # ---- input loads: z's operands first, chunk-interleaved ----
            nT_sb = singles.tile([P, NC_F, N], f32, name="nT_sb")
            w_sb = singles.tile([P, NC_F, D], f32, name="w_sb")
            for cf in range(NC_F):
                nc.sync.dma_start(out=nT_sb[:, cf, :], in_=nT_r[:, cf, :])
                nc.sync.dma_start(out=w_sb[:, cf, :], in_=w_r[:, cf, :])
            atop_b = singles.tile([P, D], f32, name="atop_b")
            nc.sync.dma_start(out=atop_b, in_=atop.to_broadcast([P, D]))
            abot_b = singles.tile([P, D], f32, name="abot_b")
            nc.sync.dma_start(out=abot_b, in_=abot.to_broadcast([P, D]))
            ident = singles.tile([P, P], f32, name="ident")
            make_identity(nc, ident)
            negc = singles.tile([P, 1], f32, name="negc")
            nc.vector.memset(negc, NEG)

            # edge-group stream loads (issued early, consumed after z)
            cb_sb = singles.tile([P, NG, SG, P], fp8, name="cb_sb")
            nb_sb = singles.tile([P, NG, SG, N], fp8, name="nb_sb")
            for r in range(NC_H):
                nc.sync.dma_start(
                    out=cb_sb[:, r],
                    in_=ch[r].rearrange("(s p) c -> p s c", p=P))
                nc.sync.dma_start(
                    out=nb_sb[:, r],
                    in_=nf[r].rearrange("(s p) c -> p s c", p=P))

            # ---- z = nodes @ w (fp32), contraction pass outermost so the
            # first matmul only needs the first nT/w chunks ----
            z_sb = singles.tile([P, NC_N, D], f32, name="z_sb")
            u_sb = singles.tile([P, NC_H], f32, name="u_sb")
            wb_sb = singles.tile([P, NC_F], f32, name="wb_sb")
            pT_all = singles.tile([P, NC_H, N], f32, name="pT_all")
            v_row = singles.tile([1, N], f32, name="v_row")
            V_bc = singles.tile([P, N], f32, name="V_bc")
            with tc.tile_pool(name="uscr", bufs=2) as uscr:
                # ---- z rows 0..511 + v, using PSUM banks 0..3 ----
                with tc.tile_pool(name="zA_ps", bufs=1,
                                  space="PSUM") as zA_ps:
                    zpA = [zA_ps.tile([P, D], f32, name=f"zp_{cn}",
                                      tag=f"zp_{cn}") for cn in range(4)]
                    for cf in range(NC_F):
                        for cn in range(4):
                            nc.tensor.matmul(
                                zpA[cn],
                                lhsT=nT_sb[:, cf, cn * P:(cn + 1) * P],
                                rhs=w_sb[:, cf, :],
                                start=(cf == 0), stop=(cf == NC_F - 1))
                    # wb[f] = sum_d w[f,d] * a_bot[d] (overlaps z matmuls)
                    for cf in range(NC_F):
                        ws = uscr.tile([P, D], f32, name=f"ws_{cf}", tag="us")
                        nc.vector.tensor_mul(ws, w_sb[:, cf, :], abot_b)
                        nc.vector.tensor_reduce(
                            wb_sb[:, cf:cf + 1], ws,
                            axis=mybir.AxisListType.X, op=Alu.add)
                    # v[m] = sum_f nodes[m,f] * wb[f]
                    for jm in range(2):
                        vp = zA_ps.tile([1, 512], f32, name=f"vp_{jm}",
                                        tag=f"zp_{jm}")
                        for cf in range(NC_F):
                            nc.tensor.matmul(
                                vp,
                                lhsT=wb_sb[:, cf:cf + 1],
                                rhs=nT_sb[:, cf, jm * 512:(jm + 1) * 512],
                                start=(cf == 0), stop=(cf == NC_F - 1))
                        nc.vector.tensor_copy(
                            v_row[:, jm * 512:(jm + 1) * 512], vp)
                    nc.gpsimd.partition_broadcast(V_bc, v_row)
                    for cn in range(4):
                        if cn % 2 == 0:
                            nc.vector.tensor_copy(z_sb[:, cn, :], zpA[cn])
                        else:
                            nc.scalar.copy(z_sb[:, cn, :], zpA[cn])
                        # u[n] = sum_d z[n,d] * a_top[d]
                        us = uscr.tile([P, D], f32, name=f"us_{cn}",
                                       tag="us")
                        nc.vector.tensor_mul(us, z_sb[:, cn, :], atop_b)
                        nc.vector.tensor_reduce(
                            u_sb[:, cn:cn + 1], us,
                            axis=mybir.AxisListType.X, op=Alu.add)
                        # pT = leaky(u[n] + v[m]) for this row chunk
                        r = cn
                        t_uv = uscr.tile([P, N], f32, name=f"tuv_{r}",
                                         tag="tuv")
                        nc.scalar.activation(t_uv, V_bc, Act.Identity,
                                             bias=u_sb[:, r:r + 1],
                                             scale=1.0)
                        nc.vector.scalar_tensor_tensor(
                            out=pT_all[:, r, :], in0=t_uv, scalar=ALPHA,
                            in1=t_uv, op0=Alu.mult, op1=Alu.max)

                # ---- z rows 512..1023 on the other 4 PSUM banks; the
                # adjacency pool coexists on the banks zA freed, so the
                # adjacency matmuls follow the z matmuls back-to-back ----
                adj_ps = tc.alloc_tile_pool(name="adj_ps", bufs=2,
                                            space="PSUM")
                zB_ps = tc.alloc_tile_pool(name="zB_ps", bufs=1,
                                           space="PSUM")
                zpB = [zB_ps.tile([P, D], f32, name=f"zp_{cn}",
                                  tag=f"zp_{cn}") for cn in range(4, NC_N)]
                for cf in range(NC_F):
                    for cn in range(4, NC_N):
                        nc.tensor.matmul(
                            zpB[cn - 4],
                            lhsT=nT_sb[:, cf, cn * P:(cn + 1) * P],
                            rhs=w_sb[:, cf, :],
                            start=(cf == 0), stop=(cf == NC_F - 1))
                for cn in range(4, NC_N):
                    if cn % 2 == 0:
                        nc.vector.tensor_copy(z_sb[:, cn, :], zpB[cn - 4])
                    else:
                        nc.scalar.copy(z_sb[:, cn, :], zpB[cn - 4])
                zB_ps.release()

            # ---- adjacency matmul + softmax + transpose + out, per r ----
            # adj row-chunk r only needs edge group r (grouped by source).
            # logits = adj*pT + (adj-1)*1e9
            # (exact: for adj==1 the +(adj-1)*1e9 term is exactly 0)
            # softmax pipeline runs in m-halves (j = 0/1) to shorten the
            # serial chain; adjacency stays resident in PSUM.
            eT_sb = singles.tile([P, NC_N, H], f32, name="eT_sb")
            out_r = out.rearrange("(r p) d -> p r d", p=P)

            def emit_adj(r):
                pj = []
                for j in range(2):
                    apj = adj_ps.tile([P, 512], f32,
                                      name=f"adj_{r}_{j}", tag=f"adj_{j}")
                    pj.append(apj)
                    for t in range(SG // 2):
                        ks = slice(2 * t, 2 * t + 2)
                        nc.tensor.matmul(
                            apj,
                            lhsT=cb_sb[:, r, ks, :],
                            rhs=nb_sb[:, r, ks, j * 512:(j + 1) * 512],
                            start=(t == 0), stop=(t == SG // 2 - 1),
                            perf_mode=DR)
                return pj

            # software pipeline: adjacency for row chunks r and r+1 in
            # flight while chunk r-2's softmax/transpose/matmul drain
            adj_tiles = {0: emit_adj(0), 1: emit_adj(1)}
            with tc.tile_pool(name="pscr", bufs=2) as pscr, \
                 tc.tile_pool(name="sml", bufs=6) as sml, \
                 tc.tile_pool(name="tp2_ps", bufs=2, space="PSUM") as tp2_ps, \
                 tc.tile_pool(name="o_ps", bufs=2, space="PSUM") as o_ps, \
                 tc.tile_pool(name="oscr", bufs=2) as oscr:
                for r in range(NC_H):
                    adj_pj = adj_tiles.pop(r)
                    # softmax over m, pipelined in halves (reads PSUM)
                    m1 = pscr.tile([P, N], f32, name=f"m1_{r}", tag="m1")
                    s19 = pscr.tile([P, N], f32, name=f"s19_{r}", tag="s19")
                    lg = pscr.tile([P, N], f32, name=f"lg_{r}", tag="lg")
                    nmh = sml.tile([P, 2], f32, name=f"nmh_{r}", tag="nmh")
                    for j in range(2):
                        sl = slice(j * 512, (j + 1) * 512)
                        nc.vector.tensor_mul(m1[:, sl], adj_pj[j],
                                             pT_all[:, r, sl])
                        # (adj-1)*1e9 via ACT free affine (fma: exact)
                        nc.scalar.activation(s19[:, sl], adj_pj[j],
                                             Act.Identity,
                                             bias=negc[:, 0:1], scale=-NEG)
                        nc.vector.tensor_add(lg[:, sl], m1[:, sl], s19[:, sl])
                        nc.vector.tensor_reduce(
                            nmh[:, j:j + 1], lg[:, sl],
                            axis=mybir.AxisListType.X, op=Alu.max,
                            negate=True)
                    nmx = sml.tile([P, 1], f32, name=f"nmx_{r}", tag="nmx")
                    # -max(a,b) = min(-a,-b)
                    nc.vector.tensor_tensor(nmx, nmh[:, 0:1], nmh[:, 1:2],
                                            op=Alu.min)
                    e_t = pscr.tile([P, N], f32, name=f"et_{r}", tag="et")
                    ea = pscr.tile([P, N], f32, name=f"ea_{r}", tag="ea")
                    zh = sml.tile([P, 2], f32, name=f"zh_{r}", tag="zh")
                    for j in range(2):
                        sl = slice(j * 512, (j + 1) * 512)
                        nc.scalar.activation(e_t[:, sl], lg[:, sl], Act.Exp,
                                             bias=nmx[:, 0:1], scale=1.0,
                                             accum_out=zh[:, j:j + 1])
                        nc.vector.tensor_mul(ea[:, sl], e_t[:, sl], adj_pj[j])
                    zr = sml.tile([P, 1], f32, name=f"zr_{r}", tag="zr")
                    nc.vector.tensor_add(zr, zh[:, 0:1], zh[:, 1:2])
                    rcp = sml.tile([P, 1], f32, name=f"rcp_{r}", tag="rcp")
                    nc.vector.reciprocal(rcp, zr)

                    if r + 2 < NC_H:
                        adj_tiles[r + 2] = emit_adj(r + 2)

                    # transpose expadj chunk into eT columns r*128..
                    for cm in range(NC_N):
                        pt2 = tp2_ps.tile([P, P], f32, name=f"tp2_{r}_{cm}",
                                          tag="tp2")
                        nc.tensor.transpose(
                            pt2, ea[:, cm * P:(cm + 1) * P], ident)
                        if cm % 3 == 0:
                            nc.vector.tensor_copy(
                                eT_sb[:, cm, r * P:(r + 1) * P], pt2)
                        else:
                            nc.scalar.copy(
                                eT_sb[:, cm, r * P:(r + 1) * P], pt2)

                    # out chunk = leaky(rcp * (expadj^T)^T @ z)
                    op = o_ps.tile([P, D], f32, name=f"op_{r}", tag="op")
                    for cm in range(NC_N):
                        nc.tensor.matmul(
                            op,
                            lhsT=eT_sb[:, cm, r * P:(r + 1) * P],
                            rhs=z_sb[:, cm, :],
                            start=(cm == 0), stop=(cm == NC_N - 1))
                    o_t = oscr.tile([P, D], f32, name=f"ot_{r}", tag="ot")
                    nc.scalar.mul(o_t, op, rcp[:, 0:1])
                    o_l = oscr.tile([P, D], f32, name=f"ol_{r}", tag="ol")
                    nc.vector.scalar_tensor_tensor(
                        out=o_l, in0=o_t, scalar=ALPHA, in1=o_t,
                        op0=Alu.mult, op1=Alu.max)
                    nc.sync.dma_start(out=out_r[:, r, :], in_=o_l)
            adj_ps.release()

    nc.compile()
    return nc


def _get_compiled():
    global _compiled
    if _compiled is None:
        _compiled = _build()
    return _compiled


def _in_maps(nodes, Cmat, Nmat, w, attention):
    import ml_dtypes
    f8 = ml_dtypes.float8_e4m3
    nodes = np.asarray(nodes, dtype=np.float32)
    Cmat = np.asarray(Cmat, dtype=np.float32)
    Nmat = np.asarray(Nmat, dtype=np.float32)
    w = np.ascontiguousarray(np.asarray(w, dtype=np.float32))
    attention = np.asarray(attention, dtype=np.float32)
    atop = np.ascontiguousarray(attention[:D, 0][None, :])
    abot = np.ascontiguousarray(attention[D:, 0][None, :])
    maps = []
    for core in range(N_CORES):
        b, h = divmod(core, 2)
        lo, hi = h * H, (h + 1) * H
        src = Cmat[b].argmax(axis=1)
        ch_g = np.zeros((NG, EPG, P), dtype=f8)
        nf_g = np.zeros((NG, EPG, N), dtype=f8)
        for r in range(NG):
            glo = lo + r * P
            sel = np.nonzero((src >= glo) & (src < glo + P))[0]
            assert len(sel) <= EPG, f"group overflow: {len(sel)} > {EPG}"
            ch_g[r, :len(sel)] = Cmat[b][sel][:, glo:glo + P].astype(f8)
            nf_b = Nmat[b][sel]
            if h == 0:
                nf_g[r, :len(sel)] = nf_b.astype(f8)
            else:
                nf_g[r, :len(sel), :H] = nf_b[:, lo:hi].astype(f8)
                nf_g[r, :len(sel), H:] = nf_b[:, :lo].astype(f8)
        if h == 0:
            nodesT = nodes[b].T
        else:
            nodesT = np.concatenate([nodes[b, lo:hi], nodes[b, :lo]],
                                    axis=0).T
        maps.append({
            "ch": ch_g,
            "nf": nf_g,
            "nodesT": np.ascontiguousarray(nodesT),
            "w": w,
            "atop": atop,
            "abot": abot,
        })
    return maps


def kernel(nodes, Cmat, Nmat, mask, w, attention, _trace=False, _tmpdir=None):
    from concourse.bass_utils import run_bass_kernel_spmd

    nc = _get_compiled()
    maps = _in_maps(nodes, Cmat, Nmat, w, attention)
    res = run_bass_kernel_spmd(nc, maps, list(range(N_CORES)),
                               trace=_trace, tmpdir=_tmpdir)
    full = np.empty((B, N, D), dtype=np.float32)
    for core in range(N_CORES):
        b, h = divmod(core, 2)
        full[b, h * H:(h + 1) * H, :] = res.results[core]["out"]
    if _trace:
        return full, res
    return full


if __name__ == "__main__":
    rng = np.random.default_rng(0)
    src = rng.integers(0, N, (B, E))
    dst = rng.integers(0, N, (B, E))
    Cm = np.eye(N, dtype=np.float32)[src]
    Nm = np.eye(N, dtype=np.float32)[dst]
    nodes = rng.standard_normal((B, N, F)).astype(np.float32)
    w = (rng.standard_normal((F, D)) * 0.05).astype(np.float32)
    att = (rng.standard_normal((2 * D, 1)) * 0.05).astype(np.float32)
    mask = np.ones((B, N, N), dtype=bool)
    got = kernel(nodes, Cm, Nm, mask, w, att)
    print("kernel ran, output shape", got.shape)
